# revision 46
# baseline (speedup 1.0000x reference)
"""Grouped gated DeltaNet (KDA-style) on 8 TRN2 NeuronCores.

Sharding: core c -> (batch b = c//4, head-group hg = c%4 of 4 heads).
Per core: column-sharded projections (weights resident, loaded once),
short-conv+silu, l2norm, chunked gated delta-rule recurrence (chunk
C=128, group decay via 1-partition f32r broadcast matmuls + fused
sub/clamp, 16-partition group correlation matmuls, transpose-free A/B
doubling with interleaved triangular-solve application), deferred gated
RMSNorm (batched over T), row-shard output projection. Host sums 4
partials per batch.

Self-contained: B=2, T=1024, D=2048, H=16, DK=DV=128 hardcoded.
"""
import sys
sys.path.insert(0, '/opt/trn_rl_repo')
import numpy as np
import ml_dtypes
from contextlib import ExitStack

B, T, D = 2, 1024, 2048
H, DK, DV, GG = 16, 128, 128, 16
NG = DK // GG          # 8 gate groups per head
NH = 4                 # heads per core
C = 128                # chunk length
NCH = T // C
SCALE = DK ** -0.5
EPS = 1e-5

# packf fp32 column offsets
PF_CW = 0        # 4 blocks x 12
PF_BG = 48
PF_NW = 52
PF_NEGA = 53     # [8,4] (n,h)
PF_DTB = 57      # [8,4]
PF_SC8 = 61
PF_EPS8 = 62
PF_EPSN = 63
PF_REPL = 64     # [8,128]
PF_IDF = 192     # [128,128]
PF_SEL = 320     # [8,1024] group-selector
PF_GMC = 1344    # [128,8] group row-mask cols
NF = 1352
# packb bf16 column offsets
PB_OH8 = 0       # [128,64]
PB_S8B = 64      # [8,1024]
PB_MM = 1088     # [128,128]
PB_MG = 1216
PB_IDB = 1344
PB_OCT = 1472    # [128,1]
PB_O1B = 1473    # [1,128]
NB = 1601

WQ0, WK0, WV0, WG0, WF10, WB0 = 0, 512, 1024, 1536, 2048, 2176
WALLC = 2180

BF = ml_dtypes.bfloat16
_CACHE = {}

FP32_CHAIN = False   # fp32 x-chain fallback (precision)


def _build():
    import concourse.tile as tile
    from concourse import bacc, mybir

    fp32 = mybir.dt.float32
    f32r = mybir.dt.float32r
    bf16 = mybir.dt.bfloat16
    Alu = mybir.AluOpType
    Act = mybir.ActivationFunctionType

    nc = bacc.Bacc("TRN2", target_bir_lowering=False, debug=False, num_devices=8)
    dp = lambda n, sh, dt: nc.dram_tensor(n, sh, dt, kind="ExternalInput").ap()
    hT = dp("hT", [D, T], bf16)
    wall = dp("wall", [D, WALLC], bf16)
    wo = dp("wo", [NH * DV, D], bf16)
    wf2 = dp("wf2", [DV, NH * NG], bf16)
    packf = dp("packf", [128, NF], fp32)
    packb = dp("packb", [128, NB], bf16)
    outT = nc.dram_tensor("outT", [D, T], fp32, kind="ExternalOutput").ap()

    with tile.TileContext(nc) as tc, ExitStack() as ctx:
        pool = lambda name, bufs, space="SBUF": ctx.enter_context(
            tc.tile_pool(name=name, bufs=bufs, space=space))

        cons = pool("cons", 1)
        pers = pool("pers", 1)
        stp = pool("st", 1)

        dma = nc.sync.dma_start

        pf = cons.tile([128, NF], fp32, tag="packf", name="packf")
        dma(pf[:], packf[:])
        pb = cons.tile([128, NB], bf16, tag="packb", name="packb")
        dma(pb[:], packb[:])
        wf2t = cons.tile([128, 32], bf16, tag="wf2t", name="wf2t")
        dma(wf2t[:], wf2[:])

        cwt = lambda m: pf[:, PF_CW + m * 12: PF_CW + (m + 1) * 12]
        bgt = pf[:, PF_BG:PF_BG + 4]
        nwt = pf[:, PF_NW:PF_NW + 1]
        negat8 = lambda h: pf[0:8, PF_NEGA + h:PF_NEGA + h + 1]
        dtbt = pf[0:8, PF_DTB:PF_DTB + 4]
        sc8t = pf[0:8, PF_SC8:PF_SC8 + 1]
        eps8t = pf[0:8, PF_EPS8:PF_EPS8 + 1]
        epsnt = pf[0:1, PF_EPSN:PF_EPSN + 1]
        replt = pf[0:8, PF_REPL:PF_REPL + 128]
        idf = pf[:, PF_IDF:PF_IDF + 128]
        sel8 = pf[0:8, PF_SEL:PF_SEL + 1024]
        oh8t = pb[:, PB_OH8:PB_OH8 + 64]
        s8b = pb[0:8, PB_S8B:PB_S8B + 1024]
        mMt = pb[:, PB_MM:PB_MM + 128]
        mGt = pb[:, PB_MG:PB_MG + 128]
        idb = pb[:, PB_IDB:PB_IDB + 128]
        octb = pb[:, PB_OCT:PB_OCT + 1]
        o1b = pb[0:1, PB_O1B:PB_O1B + 128]
        gmct = pf[:, PF_GMC:PF_GMC + 8]

        ones32 = cons.tile([32, C], fp32, tag="ones32", name="ones32")
        nc.vector.memset(ones32[:], 1.0)

        # ---- persistent activations ----
        mk = lambda nm: [pers.tile([128, T], bf16, tag=f"{nm}{m}", name=f"{nm}{m}")
                         for m in range(4)]
        qb, kb, vb = mk("qb"), mk("kb"), mk("vb")
        gateb = mk("gateb")
        f1b = pers.tile([128, T], bf16, tag="f1b", name="f1b")
        gna8 = [pers.tile([8, T], bf16, tag=f"gna{h}", name=f"gna{h}")
                for h in range(4)]
        bsg = pers.tile([4, T], fp32, tag="bsg", name="bsg")

        # ---- state tiles (parity pairs) ----
        Sf = [[stp.tile([128, DV], fp32, tag=f"Sf{h}_{p}", name=f"Sf{h}_{p}")
               for p in range(2)] for h in range(4)]
        Sb = [[stp.tile([128, DV], bf16, tag=f"Sb{h}_{p}", name=f"Sb{h}_{p}")
               for p in range(2)] for h in range(4)]
        for h in range(4):
            nc.vector.memset(Sf[h][0][:], 0.0)
            nc.vector.memset(Sb[h][0][:], 0.0)

        # ================= projections =================
        htp = ctx.enter_context(tc.tile_pool(name="htp", bufs=1))
        wallBp = ctx.enter_context(tc.tile_pool(name="wallBp", bufs=1))
        ht = [htp.tile([128, T], bf16, tag=f"ht{k}", name=f"ht{k}")
              for k in range(16)]
        wtB = [wallBp.tile([128, 512], bf16, tag=f"wB{k}", name=f"wB{k}")
               for k in range(16)]
        qs = {}

        def project(wts, pr, col0, m, dst_bf16=None, conv_slot=None, pair=None,
                    gate_bias=None, ptag="proj", pbufs=2):
                xpad = None
                if conv_slot is not None:
                    xpad = convp.tile([128, T + 3], fp32, tag="xpad", name="xpad")
                    nc.vector.memset(xpad[:, 0:3], 0.0)
                for half in range(2):
                    ps = pr.tile([128, 512], fp32, tag=ptag, name="projps", bufs=pbufs)
                    for k in range(16):
                        nc.tensor.matmul(ps[:], wts[k][:, col0 + m * 128:col0 + (m + 1) * 128],
                                         ht[k][:, half * 512:(half + 1) * 512],
                                         start=(k == 0), stop=(k == 15))
                    if xpad is not None:
                        nc.scalar.copy(xpad[:, 3 + half * 512: 3 + (half + 1) * 512], ps[:])
                    elif gate_bias is not None:
                        nc.scalar.activation(dst_bf16[:, half * 512:(half + 1) * 512],
                                             ps[:], Act.Silu, bias=gate_bias)
                    else:
                        nc.scalar.copy(dst_bf16[:, half * 512:(half + 1) * 512], ps[:])
                if xpad is None:
                    return
                cwm = cwt(m)
                s = conv_slot * 4
                a = convp.tile([128, T], fp32, tag="acca", name="acca", bufs=1)
                bt = convp.tile([128, T], fp32, tag="accb", name="accb", bufs=1)
                nc.vector.tensor_scalar(a[:], xpad[:, 3:3 + T], cwm[:, s + 3:s + 4],
                                        None, op0=Alu.mult)
                cur, nxt = a, bt
                for kk in (2, 1, 0):
                    nc.vector.scalar_tensor_tensor(nxt[:], xpad[:, kk:kk + T],
                                                   cwm[:, s + kk:s + kk + 1], cur[:],
                                                   op0=Alu.mult, op1=Alu.add)
                    cur, nxt = nxt, cur
                if pair is None:
                    nc.scalar.activation(dst_bf16[:], cur[:], Act.Silu)
                else:
                    qsil = qb[pair] if pair < 4 else kb[pair - 4]
                    qs[pair] = qsil
                    nc.scalar.activation(qsil[:], cur[:], Act.Silu)
                    sq = smt.tile([128, T], bf16, tag="sq", name="sq", bufs=1)
                    nc.scalar.activation(sq[:], qsil[:], Act.Square)
                    for half in range(2):
                        pss = pr.tile([8, 512], fp32, tag="sqs", name="sqs")
                        nc.tensor.matmul(pss[:], oh8t[:, pair * 8:pair * 8 + 8],
                                         sq[:, half * 512:(half + 1) * 512],
                                         start=True, stop=True)
                        nc.vector.tensor_tensor(ssqsb[:, half * 512:(half + 1) * 512],
                                                ssqsb[:, half * 512:(half + 1) * 512],
                                                pss[:], op=Alu.add)

        with tc.tile_pool(name="wallAp", bufs=1) as wallAp, \
             tc.tile_pool(name="convp", bufs=2) as convp, \
             tc.tile_pool(name="smt", bufs=2) as smt, \
             tc.tile_pool(name="pps", bufs=1, space="PSUM") as pr:
            # PSUM tags: proj(2) sqs(1) bps(1) gps(1) nb(2) = 7 banks
            ssqsb = smt.tile([8, T], fp32, tag="ssqsb", name="ssqsb", bufs=1)
            nc.vector.memset(ssqsb[:], 0.0)
            wtA = []
            for k in range(16):
                dma(ht[k][:], hT[k * 128:(k + 1) * 128, :])
                wA = wallAp.tile([128, 1668], bf16, tag=f"wA{k}", name=f"wA{k}")
                dma(wA[:, 0:1536], wall[k * 128:(k + 1) * 128, 0:1536])
                dma(wA[:, 1536:1668], wall[k * 128:(k + 1) * 128, WF10:WF10 + 132])
                wtA.append(wA)
            for k in range(16):
                dma(wtB[k][:], wall[k * 128:(k + 1) * 128, WG0:WG0 + 512])
            for m in range(4):
                project(wtA, pr, 0, m, conv_slot=0, pair=m)
            for m in range(4):
                project(wtA, pr, 512, m, conv_slot=1, pair=4 + m)
            for m in range(4):
                project(wtA, pr, 1024, m, dst_bf16=vb[m], conv_slot=2)

            # f1 projection
            for half in range(2):
                ps = pr.tile([128, 512], fp32, tag="proj", name="f1ps", bufs=2)
                for k in range(16):
                    nc.tensor.matmul(ps[:], wtA[k][:, 1536:1664],
                                     ht[k][:, half * 512:(half + 1) * 512],
                                     start=(k == 0), stop=(k == 15))
                nc.scalar.copy(f1b[:, half * 512:(half + 1) * 512], ps[:])

            # beta (sigmoid) then gate-softplus chain, table-load friendly order
            for half in range(2):
                bps = pr.tile([4, 512], fp32, tag="bps", name="bps")
                for k in range(16):
                    nc.tensor.matmul(bps[:], wtA[k][:, 1664:1668],
                                     ht[k][:, half * 512:(half + 1) * 512],
                                     start=(k == 0), stop=(k == 15))
                nc.scalar.activation(bsg[:, half * 512:(half + 1) * 512], bps[:],
                                     Act.Sigmoid)
            sp1s = []
            for half in range(2):
                for h in range(4):
                    gps = pr.tile([8, 512], fp32, tag="gps", name="gps", bufs=2)
                    nc.tensor.matmul(gps[:], wf2t[:, h * 8:(h + 1) * 8],
                                     f1b[:, half * 512:(half + 1) * 512],
                                     start=True, stop=True)
                    spe = smt.tile([8, 512], bf16, tag="spe", name="spe", bufs=2)
                    nc.scalar.activation(spe[:], gps[:], Act.Exp,
                                         bias=dtbt[:, h:h + 1])
                    sp1 = smt.tile([8, 512], bf16, tag="sp1", name="sp1", bufs=8)
                    nc.vector.tensor_scalar(sp1[:], spe[:], 1.0, None, op0=Alu.add)
                    sp1s.append((half, h, sp1))
            # all Ln together: l2 normalizer + softplus logs
            nrm = smt.tile([8, T], fp32, tag="nrm", name="nrm", bufs=1)
            nc.scalar.activation(nrm[:], ssqsb[:], Act.Ln, scale=sc8t[:, 0:1],
                                 bias=eps8t[:, 0:1])
            for half, h, sp1 in sp1s:
                sp = smt.tile([8, 512], bf16, tag="sp", name="sp", bufs=2)
                nc.scalar.activation(sp[:], sp1[:], Act.Ln)
                nc.vector.tensor_scalar(gna8[h][:, half * 512:(half + 1) * 512],
                                        sp[:], negat8(h), None, op0=Alu.mult)
            recb = smt.tile([8, T], bf16, tag="recb", name="recb", bufs=1)
            nc.scalar.activation(recb[:], nrm[:], Act.Exp, scale=-0.5)
            for pair in range(8):
                dst = qb[pair] if pair < 4 else kb[pair - 4]
                for half in range(2):
                    nb = pr.tile([128, 512], fp32, tag="nb", name="nb", bufs=2)
                    nc.tensor.matmul(nb[:], s8b[:, pair * 128:(pair + 1) * 128],
                                     recb[:, half * 512:(half + 1) * 512],
                                     start=True, stop=True)
                    nc.vector.tensor_tensor(dst[:, half * 512:(half + 1) * 512],
                                            qs[pair][:, half * 512:(half + 1) * 512],
                                            nb[:], op=Alu.mult)

        pers2 = ctx.enter_context(tc.tile_pool(name="pers2", bufs=1))
        yb = [pers2.tile([128, T], bf16, tag=f"yb{m}", name=f"yb{m}")
              for m in range(4)]
        wotp = ctx.enter_context(tc.tile_pool(name="wotp", bufs=1))
        wot = [wotp.tile([128, D], bf16, tag=f"wo{k}", name=f"wo{k}") for k in range(4)]

        # ================= recurrence =================
        rctx = ExitStack()
        rc = rctx.enter_context(tc.tile_pool(name="rc", bufs=2))
        rr = rctx.enter_context(tc.tile_pool(name="rr", bufs=3))
        prc = rctx.enter_context(tc.tile_pool(name="prc", bufs=1, space="PSUM"))
        # PSUM tags: tp(2) big(2) dblx(2) mx(2) = 8 banks

        hdt = lambda nm, h, sh, dt=bf16, bufs=2: rc.tile(
            sh, dt, tag=f"{nm}{h}", name=f"{nm}{h}", bufs=bufs)

        MSKN = {1: 0, 3: 1, 5: 2, 6: 3, 7: 4}

        def pro_vt(P, early=False):
            ts = P['ts']
            P['vt'], P['kts'] = [], []
            for h in range(4):
                vtp = prc.tile([128, C], bf16, tag="tp", name="vtp", bufs=2)
                nc.tensor.transpose(vtp[:], vb[h][:, ts], idb[:])
                vt = hdt("vt", h, [128, C])
                if early:
                    nc.vector.tensor_copy(vt[:], vtp[:])
                else:
                    nc.scalar.copy(vt[:], vtp[:])
                P['vt'].append(vt)
                ktp = prc.tile([128, C], bf16, tag="tp", name="ktp", bufs=2)
                nc.tensor.transpose(ktp[:], P['_kends'][h][:], idb[:])
                kts = hdt("kts", h, [128, C])
                nc.vector.tensor_scalar(kts[:], ktp[:], P['_beta2'][:, h:h + 1],
                                        None, op0=Alu.mult)
                P['kts'].append(kts)

        def prologue(ci, with_vt=True, early=False):
            ts = slice(ci * C, (ci + 1) * C)
            P = {'ts': ts}
            cn8s = []
            for h in range(4):
                cn8 = hdt("cn8", h, [8, C], fp32)
                nc.vector.tensor_tensor_scan(cn8[:], ones32[0:8, :],
                                             gna8[h][:, ts], 0.0,
                                             op0=Alu.mult, op1=Alu.add)
                cn8s.append(cn8)
            cnt8s = []
            for h in range(4):
                cNtp = prc.tile([128, 8], fp32, tag="tp", name="cNtp", bufs=2)
                nc.tensor.transpose(cNtp[:], cn8s[h][:], idf[0:8, 0:8])
                cnt8 = hdt("cnt8", h, [128, 8], fp32)
                nc.scalar.copy(cnt8[:], cNtp[:])
                cnt8s.append(cnt8)
            b2p = prc.tile([128, 4], fp32, tag="tp", name="b2p", bufs=2)
            nc.tensor.transpose(b2p[:], bsg[:, ts], idf[0:4, 0:4])
            beta2 = rc.tile([128, 4], fp32, tag="beta2", name="beta2")
            nc.scalar.copy(beta2[:], b2p[:])
            kmsks = []
            for h in range(4):
                kmsk = rr.tile([128, 5 * C], bf16, tag="kmsk", name="kmsk")
                for n, j in MSKN.items():
                    dst = kmsk[:, j * C:(j + 1) * C]
                    if j < 3:
                        nc.scalar.mul(dst, kb[h][:, ts], gmct[:, n:n + 1])
                    else:
                        nc.vector.tensor_scalar(dst, kb[h][:, ts], gmct[:, n:n + 1],
                                                None, op0=Alu.mult)
                kmsks.append(kmsk)
            exp8s, exp8ks = [], []
            for h in range(4):
                e8 = hdt("exp8", h, [8, C], fp32)
                nc.scalar.activation(e8[:], cn8s[h][:], Act.Exp)
                exp8s.append(e8)
            for h in range(4):
                e8k = hdt("exp8k", h, [8, C], fp32)
                nc.scalar.activation(e8k[:], cn8s[h][:], Act.Exp, scale=-1.0,
                                     bias=cn8s[h][:, C - 1:C])
                exp8ks.append(e8k)
            bfks = []
            for h in range(4):
                bfk = prc.tile([128, 256], fp32, tag="tp", name="bfk", bufs=2)
                nc.tensor.matmul(bfk[:, 0:128], replt, exp8s[h][:],
                                 start=True, stop=True)
                nc.tensor.matmul(bfk[:, 128:256], replt, exp8ks[h][:],
                                 start=True, stop=True)
                bfks.append(bfk)
            P['bC'] = []
            for h in range(4):
                bC = hdt("bC", h, [128, 1], fp32)
                nc.scalar.copy(bC[:], bfks[h][:, 127:128])
                P['bC'].append(bC)
            P['Wt'], P['qtT'], kends = [], [], []
            for h in range(4):
                Wth = hdt("Wt", h, [128, C])
                nc.vector.tensor_tensor(Wth[:], kb[h][:, ts], bfks[h][:, 0:128],
                                        op=Alu.mult)
                P['Wt'].append(Wth)
                qtTh = hdt("qtT", h, [128, C])
                nc.vector.tensor_tensor(qtTh[:], qb[h][:, ts], bfks[h][:, 0:128],
                                        op=Alu.mult)
                P['qtT'].append(qtTh)
                kendh = hdt("kend", h, [128, C])
                nc.vector.tensor_tensor(kendh[:], kb[h][:, ts], bfks[h][:, 128:256],
                                        op=Alu.mult)
                kends.append(kendh)
            ealls = []
            for h in range(4):
                bcaL = prc.tile([128, 512], fp32, tag="big", name="bcaL", bufs=3)
                bcaH = prc.tile([128, 512], fp32, tag="big", name="bcaH", bufs=3)
                for n in range(8):
                    dst = bcaL if n < 4 else bcaH
                    nc.tensor.matmul(dst[:, (n % 4) * C:(n % 4 + 1) * C],
                                     sel8[:, n * 128:(n + 1) * 128], cn8s[h][:],
                                     start=True, stop=True)
                eallin = rr.tile([128, 8 * C], bf16, tag="eallin", name="eallin")
                nrelu = 0 if early else 4
                for n in range(nrelu):
                    nc.scalar.activation(eallin[:, n * C:(n + 1) * C],
                                         bcaL[:, n * C:(n + 1) * C],
                                         Act.Relu, scale=-1.0,
                                         bias=cnt8s[h][:, n:n + 1])
                for n in range(nrelu, 8):
                    bsrc = bcaL if n < 4 else bcaH
                    nc.vector.tensor_scalar(eallin[:, n * C:(n + 1) * C],
                                            bsrc[:, (n % 4) * C:(n % 4 + 1) * C],
                                            cnt8s[h][:, n:n + 1], 0.0,
                                            op0=Alu.subtract, op1=Alu.min)
                eall = rr.tile([128, 8 * C], bf16, tag="eall", name="eall", bufs=4)
                if nrelu:
                    nc.scalar.activation(eall[:, 0:nrelu * C], eallin[:, 0:nrelu * C],
                                         Act.Exp, scale=-1.0)
                nc.scalar.activation(eall[:, nrelu * C:], eallin[:, nrelu * C:],
                                     Act.Exp)
                ealls.append(eall)
            P['A'] = [[None] * 7 for _ in range(4)]
            P['GtM'] = []
            for h in range(4):
                pls = []
                for csrc in (kb[h], qb[h]):
                    pl = prc.tile([128, 512], fp32, tag="big", name="pall", bufs=3)
                    ph = prc.tile([128, 512], fp32, tag="big", name="pallh", bufs=3)
                    for n in range(8):
                        dst = pl if n < 4 else ph
                        if n in (0, 2, 4):
                            nc.tensor.matmul(dst[:, (n % 4) * C:(n % 4 + 1) * C],
                                             kb[h][16 * n:16 * (n + 1), ts],
                                             csrc[16 * n:16 * (n + 1), ts],
                                             start=True, stop=True)
                        else:
                            j = MSKN[n]
                            nc.tensor.matmul(dst[:, (n % 4) * C:(n % 4 + 1) * C],
                                             kmsks[h][:, j * C:(j + 1) * C],
                                             csrc[:, ts],
                                             start=True, stop=True)
                    pls.append((pl, ph))
                prods = []
                for x, (pl, ph) in enumerate(pls):
                    prod = rr.tile([128, 8 * C], bf16, tag="prod", name=f"prod{x}", bufs=4)
                    nc.vector.tensor_tensor(prod[:, 0:4 * C], ealls[h][:, 0:4 * C],
                                            pl[:], op=Alu.mult)
                    nc.vector.tensor_tensor(prod[:, 4 * C:], ealls[h][:, 4 * C:],
                                            ph[:], op=Alu.mult)
                    prods.append(prod)
                for x, prod in enumerate(prods):
                    # sum the 8 group blocks on PE (identity-lhsT accumulation)
                    t1 = prc.tile([128, C], fp32, tag="dblx", name="t1p", bufs=2)
                    for n in range(8):
                        nc.tensor.matmul(t1[:], idb, prod[:, n * C:(n + 1) * C],
                                         start=(n == 0), stop=(n == 7))
                    if x == 0:
                        A0 = rc.tile([128, C], bf16, tag=f"A{h}", name=f"A{h}_0",
                                     bufs=12)
                        nc.vector.scalar_tensor_tensor(A0[:], t1[:],
                                                       beta2[:, h:h + 1], mMt[:],
                                                       op0=Alu.mult, op1=Alu.mult)
                        P['A'][h][0] = A0
                    else:
                        GtM = hdt("GtM", h, [128, C])
                        nc.vector.scalar_tensor_tensor(GtM[:], t1[:],
                                                       beta2[:, h:h + 1], mGt[:],
                                                       op0=Alu.mult, op1=Alu.mult)
                        P['GtM'].append(GtM)
            P['_kends'] = kends
            P['_beta2'] = beta2
            if with_vt:
                pro_vt(P)
            # A/B doubling chains (state-independent)
            Bs = [[None] * 6 for _ in range(4)]
            for h in range(4):
                b0p = prc.tile([128, C], bf16, tag="tp", name="b0p", bufs=2)
                nc.tensor.transpose(b0p[:], P['A'][h][0][:], idb[:])
                B0 = rc.tile([128, C], bf16, tag=f"B{h}", name=f"B{h}_0", bufs=2)
                nc.scalar.copy(B0[:], b0p[:])
                Bs[h][0] = B0
            for lev in range(1, 7):
                for h in range(4):
                    dbl = prc.tile([128, 256], fp32, tag="dblx", name="dbl", bufs=2)
                    nc.tensor.matmul(dbl[:, 0:128], Bs[h][lev - 1][:],
                                     P['A'][h][lev - 1][:], start=True, stop=True)
                    if lev < 6:
                        nc.tensor.matmul(dbl[:, 128:256], P['A'][h][lev - 1][:],
                                         Bs[h][lev - 1][:], start=True, stop=True)
                    An = rc.tile([128, C], bf16, tag=f"A{h}", name=f"A{h}_{lev}",
                                 bufs=12)
                    nc.scalar.copy(An[:], dbl[:, 0:128])
                    P['A'][h][lev] = An
                    if lev < 6:
                        Bn = rc.tile([128, C], bf16, tag=f"B{h}",
                                     name=f"B{h}_{lev}", bufs=2)
                        if lev % 2 == 0 and not early:
                            nc.scalar.copy(Bn[:], dbl[:, 128:256])
                        else:
                            nc.vector.tensor_copy(Bn[:], dbl[:, 128:256])
                        Bs[h][lev] = Bn
            return P

        def spine(ci, P):
            ts = P['ts']
            po, pn = ci % 2, (ci + 1) % 2
            xbs = []
            for h in range(4):
                ws0 = prc.tile([128, C], fp32, tag="tp", name="ws0", bufs=2)
                nc.tensor.matmul(ws0[:], P['Wt'][h][:], Sb[h][po][:],
                                 start=True, stop=True)
                xb = rc.tile([128, C], bf16, tag=f"xb{h}", name=f"xb{h}", bufs=3)
                nc.vector.tensor_tensor(xb[:], P['vt'][h][:], ws0[:],
                                        op=Alu.subtract)
                xbs.append(xb)
            for lev in range(7):
                for h in range(4):
                    mx = prc.tile([128, C], fp32, tag="mx", name="mx", bufs=1)
                    nc.tensor.matmul(mx[:], P['A'][h][lev][:], xbs[h][:],
                                     start=True, stop=True)
                    xn = rc.tile([128, C], bf16, tag=f"xb{h}", name=f"xb{h}_{lev}",
                                 bufs=3)
                    nc.vector.tensor_tensor(xn[:], xbs[h][:], mx[:],
                                            op=(Alu.subtract if lev == 0 else Alu.add))
                    xbs[h] = xn
            for h in range(4):
                sup = prc.tile([128, DV], fp32, tag="tp", name="sup", bufs=2)
                nc.tensor.matmul(sup[:], P['kts'][h][:], xbs[h][:],
                                 start=True, stop=True)
                nc.vector.scalar_tensor_tensor(Sf[h][pn][:], Sf[h][po][:],
                                               P['bC'][h][:, 0:1], sup[:],
                                               op0=Alu.mult, op1=Alu.add)
                nc.vector.scalar_tensor_tensor(Sb[h][pn][:], Sf[h][po][:],
                                               P['bC'][h][:, 0:1], sup[:],
                                               op0=Alu.mult, op1=Alu.add)
                otp = prc.tile([128, C], fp32, tag="tp", name="otp", bufs=2)
                nc.tensor.matmul(otp[:], Sb[h][po][:], P['qtT'][h][:],
                                 start=True, stop=False)
                nc.tensor.matmul(otp[:], xbs[h][:], P['GtM'][h][:],
                                 start=False, stop=True)
                nc.vector.tensor_tensor(yb[h][:, ts], gateb[h][:, ts], otp[:],
                                        op=Alu.mult)

        pros = [None, None]
        pros[0] = prologue(0, with_vt=False, early=True)
        pros[1] = prologue(1, with_vt=False, early=True)
        # g projection emitted here: its PE work overlaps prologue 0/1
        for m in range(4):
            project(wtB, prc, 0, m, dst_bf16=gateb[m],
                    gate_bias=bgt[:, m:m + 1], ptag="mx", pbufs=1)
        for k in range(4):
            dma(wot[k][:], wo[k * 128:(k + 1) * 128, :])
        pro_vt(pros[0], early=True)
        pro_vt(pros[1], early=True)
        spine(0, pros[0])
        for ci in range(2, NCH):
            pros[ci % 2] = prologue(ci)
            spine(ci - 1, pros[(ci - 1) % 2])
        spine(NCH - 1, pros[(NCH - 1) % 2])

        rctx.close()
        # ================= deferred RMSNorm + output projection =================
        with tc.tile_pool(name="post", bufs=2) as post, \
             tc.tile_pool(name="ppc", bufs=2, space="PSUM") as ppc:
            # PSUM tags: ssp(2) rbc(2) proj(2) = 6 banks
            ysqs = []
            for h in range(4):
                ysq = post.tile([128, T], bf16, tag="ysq", name="ysq", bufs=4)
                nc.scalar.activation(ysq[:], yb[h][:], Act.Square)
                ysqs.append(ysq)
            nrcs = []
            for h in range(4):
                nrc = post.tile([1, T], fp32, tag="nrc", name="nrc", bufs=4)
                for half in range(2):
                    ssp = ppc.tile([1, 512], fp32, tag="ssp", name="ssp")
                    nc.tensor.matmul(ssp[:], octb[:],
                                     ysqs[h][:, half * 512:(half + 1) * 512],
                                     start=True, stop=True)
                    nc.scalar.activation(nrc[:, half * 512:(half + 1) * 512],
                                         ssp[:], Act.Ln, scale=1.0 / DV,
                                         bias=epsnt[:, 0:1])
                nrcs.append(nrc)
            for h in range(4):
                rcb = post.tile([1, T], bf16, tag="rcb", name="rcb", bufs=4)
                nc.scalar.activation(rcb[:], nrcs[h][:], Act.Exp, scale=-0.5)
                for half in range(2):
                    rbc = ppc.tile([128, 512], fp32, tag="rbc", name="rbc")
                    nc.tensor.matmul(rbc[:], o1b[:], rcb[:, half * 512:(half + 1) * 512],
                                     start=True, stop=True)
                    nc.vector.scalar_tensor_tensor(yb[h][:, half * 512:(half + 1) * 512],
                                                   yb[h][:, half * 512:(half + 1) * 512],
                                                   nwt[:, 0:1], rbc[:],
                                                   op0=Alu.mult, op1=Alu.mult)
            # output projection
            for m in range(16):
                osb = post.tile([128, T], fp32, tag="osb", name="osb")
                for half in range(2):
                    ps = ppc.tile([128, 512], fp32, tag="proj", name="ops")
                    for k in range(4):
                        nc.tensor.matmul(ps[:], wot[k][:, m * 128:(m + 1) * 128],
                                         yb[k][:, half * 512:(half + 1) * 512],
                                         start=(k == 0), stop=(k == 3))
                    if half == 0:
                        nc.vector.tensor_copy(osb[:, 0:512], ps[:])
                    else:
                        nc.scalar.copy(osb[:, 512:1024], ps[:])
                dma(outT[m * 128:(m + 1) * 128, :], osb[:])

    nc.compile()
    return nc


def _prep_inputs(inputs):
    f32 = np.float32
    hs = np.asarray(inputs['hidden_states'], f32)
    tri = np.tril(np.ones((C, C), f32))
    maskM = (1.0 - tri).astype(f32)
    maskG = (1.0 - tri + np.eye(C, dtype=f32)).astype(f32)
    repl = np.zeros((NG, DK), f32)
    for n in range(NG):
        repl[n, n * GG:(n + 1) * GG] = 1.0
    sel8 = np.zeros((NG, NG * 128), f32)
    for n in range(NG):
        sel8[n, n * 128:(n + 1) * 128] = 1.0
    oh8 = np.zeros((DK, 64), f32)
    for i in range(8):
        oh8[:, i * 8 + i] = 1.0
    ident = np.eye(128, dtype=f32)

    maps = []
    for c in range(8):
        b, hg = c // 4, c % 4
        cols = slice(hg * NH * DK, (hg + 1) * NH * DK)
        gcols = slice(hg * NH * NG, (hg + 1) * NH * NG)
        hcols = slice(hg * NH, (hg + 1) * NH)
        nega = -np.exp(np.repeat(np.asarray(inputs['A_log'], f32)[hcols], NG))

        packf = np.zeros((128, NF), f32)
        cw = np.concatenate(
            [np.asarray(inputs['conv_q'], f32)[cols],
             np.asarray(inputs['conv_k'], f32)[cols],
             np.asarray(inputs['conv_v'], f32)[cols]], 1)  # [512, 12]
        for m in range(4):
            packf[:, PF_CW + m * 12:PF_CW + (m + 1) * 12] = cw[m * 128:(m + 1) * 128]
        packf[:, PF_BG:PF_BG + 4] = np.asarray(inputs['bg'], f32)[cols].reshape(NH, DV).T
        packf[:, PF_NW] = np.asarray(inputs['norm_w'], f32)
        packf[0:8, PF_NEGA:PF_NEGA + 4] = nega.reshape(NH, NG).T
        packf[0:8, PF_DTB:PF_DTB + 4] = (
            np.asarray(inputs['dt_bias'], f32)[gcols].reshape(NH, NG).T)
        packf[0:8, PF_SC8] = [1.0 / SCALE ** 2] * 4 + [1.0] * 4
        packf[0:8, PF_EPS8] = [1e-6 / SCALE ** 2] * 4 + [1e-6] * 4
        packf[0:1, PF_EPSN] = EPS
        packf[0:8, PF_REPL:PF_REPL + 128] = repl
        packf[:, PF_IDF:PF_IDF + 128] = ident
        packf[0:8, PF_SEL:PF_SEL + 1024] = sel8
        packf[:, PF_GMC:PF_GMC + 8] = repl.T

        packb = np.zeros((128, NB), f32)
        packb[:, PB_OH8:PB_OH8 + 64] = oh8
        packb[0:8, PB_S8B:PB_S8B + 1024] = sel8
        packb[:, PB_MM:PB_MM + 128] = maskM
        packb[:, PB_MG:PB_MG + 128] = maskG
        packb[:, PB_IDB:PB_IDB + 128] = ident
        packb[:, PB_OCT] = 1.0
        packb[0:1, PB_O1B:PB_O1B + 128] = 1.0

        wallm = np.concatenate(
            [np.asarray(inputs['Wq'], f32)[:, cols],
             np.asarray(inputs['Wk'], f32)[:, cols],
             np.asarray(inputs['Wv'], f32)[:, cols],
             np.asarray(inputs['Wg'], f32)[:, cols],
             np.asarray(inputs['Wf1'], f32),
             np.asarray(inputs['Wb'], f32)[:, hcols]], 1)

        m = {
            'hT': np.ascontiguousarray(hs[b].T).astype(BF),
            'wall': np.ascontiguousarray(wallm).astype(BF),
            'wo': np.ascontiguousarray(np.asarray(inputs['Wo'], f32)[cols, :]).astype(BF),
            'wf2': np.ascontiguousarray(np.asarray(inputs['Wf2'], f32)[:, gcols]).astype(BF),
            'packf': packf,
            'packb': packb.astype(BF),
        }
        maps.append(m)
    return maps


def kernel(**inputs):
    from concourse.bass_utils import run_bass_kernel_spmd
    if 'nc' not in _CACHE:
        _CACHE['nc'] = _build()
    nc = _CACHE['nc']
    maps = _prep_inputs(inputs)
    res = run_bass_kernel_spmd(nc, maps, list(range(8))).results
    out = np.zeros((B, T, D), np.float32)
    for c in range(8):
        out[c // 4] += res[c]['outT'].T.astype(np.float32)
    return out


# revision 47
# speedup vs baseline: 1.0421x; 1.0421x over previous
"""Grouped gated DeltaNet (KDA-style) on 8 TRN2 NeuronCores.

Sharding: core c -> (batch b = c//4, head-group hg = c%4 of 4 heads).
Per core: column-sharded projections (weights resident, loaded once),
short-conv+silu, l2norm, chunked gated delta-rule recurrence (chunk
C=128, group decay via 1-partition f32r broadcast matmuls + fused
sub/clamp, 16-partition group correlation matmuls, transpose-free A/B
doubling with interleaved triangular-solve application), deferred gated
RMSNorm (batched over T), row-shard output projection. Host sums 4
partials per batch.

Self-contained: B=2, T=1024, D=2048, H=16, DK=DV=128 hardcoded.
"""
import sys
sys.path.insert(0, '/opt/trn_rl_repo')
import numpy as np
import ml_dtypes
from contextlib import ExitStack

B, T, D = 2, 1024, 2048
H, DK, DV, GG = 16, 128, 128, 16
NG = DK // GG          # 8 gate groups per head
NH = 4                 # heads per core
C = 128                # chunk length
NCH = T // C
SCALE = DK ** -0.5
EPS = 1e-5

# packf fp32 column offsets
PF_CW = 0        # 4 blocks x 12
PF_BG = 48
PF_NW = 52
PF_NEGA = 53     # [8,4] (n,h)
PF_DTB = 57      # [8,4]
PF_SC8 = 61
PF_EPS8 = 62
PF_EPSN = 63
PF_REPL = 64     # [8,128]
PF_IDF = 192     # [128,128]
PF_SEL = 320     # [8,1024] group-selector
PF_GMC = 1344    # [128,8] group row-mask cols
NF = 1352
# packb bf16 column offsets
PB_OH8 = 0       # [128,64]
PB_S8B = 64      # [8,1024]
PB_MM = 1088     # [128,128]
PB_MG = 1216
PB_IDB = 1344
PB_OCT = 1472    # [128,1]
PB_O1B = 1473    # [1,128]
NB = 1601

WQ0, WK0, WV0, WG0, WF10, WB0 = 0, 512, 1024, 1536, 2048, 2176
WALLC = 2180

BF = ml_dtypes.bfloat16
_CACHE = {}

FP32_CHAIN = False   # fp32 x-chain fallback (precision)


def _build():
    import concourse.tile as tile
    from concourse import bacc, mybir

    fp32 = mybir.dt.float32
    f32r = mybir.dt.float32r
    bf16 = mybir.dt.bfloat16
    Alu = mybir.AluOpType
    Act = mybir.ActivationFunctionType

    nc = bacc.Bacc("TRN2", target_bir_lowering=False, debug=False, num_devices=8)
    dp = lambda n, sh, dt: nc.dram_tensor(n, sh, dt, kind="ExternalInput").ap()
    hT = dp("hT", [D, T], bf16)
    wall = dp("wall", [D, WALLC], bf16)
    wo = dp("wo", [NH * DV, D], bf16)
    wf2 = dp("wf2", [DV, NH * NG], bf16)
    packf = dp("packf", [128, NF], fp32)
    packb = dp("packb", [128, NB], bf16)
    outT = nc.dram_tensor("outT", [D, T], fp32, kind="ExternalOutput").ap()

    with tile.TileContext(nc) as tc, ExitStack() as ctx:
        pool = lambda name, bufs, space="SBUF": ctx.enter_context(
            tc.tile_pool(name=name, bufs=bufs, space=space))

        cons = pool("cons", 1)
        pers = pool("pers", 1)
        stp = pool("st", 1)

        dma = nc.sync.dma_start

        pf = cons.tile([128, NF], fp32, tag="packf", name="packf")
        dma(pf[:], packf[:])
        pb = cons.tile([128, NB], bf16, tag="packb", name="packb")
        dma(pb[:], packb[:])
        wf2t = cons.tile([128, 32], bf16, tag="wf2t", name="wf2t")
        dma(wf2t[:], wf2[:])

        cwt = lambda m: pf[:, PF_CW + m * 12: PF_CW + (m + 1) * 12]
        bgt = pf[:, PF_BG:PF_BG + 4]
        nwt = pf[:, PF_NW:PF_NW + 1]
        negat8 = lambda h: pf[0:8, PF_NEGA + h:PF_NEGA + h + 1]
        dtbt = pf[0:8, PF_DTB:PF_DTB + 4]
        sc8t = pf[0:8, PF_SC8:PF_SC8 + 1]
        eps8t = pf[0:8, PF_EPS8:PF_EPS8 + 1]
        epsnt = pf[0:1, PF_EPSN:PF_EPSN + 1]
        replt = pf[0:8, PF_REPL:PF_REPL + 128]
        idf = pf[:, PF_IDF:PF_IDF + 128]
        sel8 = pf[0:8, PF_SEL:PF_SEL + 1024]
        oh8t = pb[:, PB_OH8:PB_OH8 + 64]
        s8b = pb[0:8, PB_S8B:PB_S8B + 1024]
        mMt = pb[:, PB_MM:PB_MM + 128]
        mGt = pb[:, PB_MG:PB_MG + 128]
        idb = pb[:, PB_IDB:PB_IDB + 128]
        octb = pb[:, PB_OCT:PB_OCT + 1]
        o1b = pb[0:1, PB_O1B:PB_O1B + 128]
        gmct = pf[:, PF_GMC:PF_GMC + 8]

        ones32 = cons.tile([32, C], fp32, tag="ones32", name="ones32")
        nc.vector.memset(ones32[:], 1.0)

        # ---- persistent activations ----
        mk = lambda nm: [pers.tile([128, T], bf16, tag=f"{nm}{m}", name=f"{nm}{m}")
                         for m in range(4)]
        qb, kb, vb = mk("qb"), mk("kb"), mk("vb")
        gateb = mk("gateb")
        f1b = pers.tile([128, T], bf16, tag="f1b", name="f1b")
        gna8 = [pers.tile([8, T], bf16, tag=f"gna{h}", name=f"gna{h}")
                for h in range(4)]
        bsg = pers.tile([4, T], fp32, tag="bsg", name="bsg")

        # ---- state tiles (parity pairs) ----
        Sf = [[stp.tile([128, DV], fp32, tag=f"Sf{h}_{p}", name=f"Sf{h}_{p}")
               for p in range(2)] for h in range(4)]
        Sb = [[stp.tile([128, DV], bf16, tag=f"Sb{h}_{p}", name=f"Sb{h}_{p}")
               for p in range(2)] for h in range(4)]
        for h in range(4):
            nc.vector.memset(Sf[h][0][:], 0.0)
            nc.vector.memset(Sb[h][0][:], 0.0)

        # ================= projections =================
        htp = ctx.enter_context(tc.tile_pool(name="htp", bufs=1))
        wallBp = ctx.enter_context(tc.tile_pool(name="wallBp", bufs=1))
        ht = [htp.tile([128, T], bf16, tag=f"ht{k}", name=f"ht{k}")
              for k in range(16)]
        wtB = [wallBp.tile([128, 512], bf16, tag=f"wB{k}", name=f"wB{k}")
               for k in range(16)]
        qs = {}

        def project(wts, pr, col0, m, dst_bf16=None, conv_slot=None, pair=None,
                    gate_bias=None, ptag="proj", pbufs=2):
                xpad = None
                if conv_slot is not None:
                    xpad = convp.tile([128, T + 3], fp32, tag="xpad", name="xpad")
                    nc.vector.memset(xpad[:, 0:3], 0.0)
                for half in range(2):
                    ps = pr.tile([128, 512], fp32, tag=ptag, name="projps", bufs=pbufs)
                    for k in range(16):
                        nc.tensor.matmul(ps[:], wts[k][:, col0 + m * 128:col0 + (m + 1) * 128],
                                         ht[k][:, half * 512:(half + 1) * 512],
                                         start=(k == 0), stop=(k == 15))
                    if xpad is not None:
                        nc.scalar.copy(xpad[:, 3 + half * 512: 3 + (half + 1) * 512], ps[:])
                    elif gate_bias is not None:
                        nc.scalar.activation(dst_bf16[:, half * 512:(half + 1) * 512],
                                             ps[:], Act.Silu, bias=gate_bias)
                    else:
                        nc.scalar.copy(dst_bf16[:, half * 512:(half + 1) * 512], ps[:])
                if xpad is None:
                    return
                cwm = cwt(m)
                s = conv_slot * 4
                a = convp.tile([128, T], fp32, tag="acca", name="acca", bufs=1)
                bt = convp.tile([128, T], fp32, tag="accb", name="accb", bufs=1)
                nc.vector.tensor_scalar(a[:], xpad[:, 3:3 + T], cwm[:, s + 3:s + 4],
                                        None, op0=Alu.mult)
                cur, nxt = a, bt
                for kk in (2, 1, 0):
                    nc.vector.scalar_tensor_tensor(nxt[:], xpad[:, kk:kk + T],
                                                   cwm[:, s + kk:s + kk + 1], cur[:],
                                                   op0=Alu.mult, op1=Alu.add)
                    cur, nxt = nxt, cur
                if pair is None:
                    nc.scalar.activation(dst_bf16[:], cur[:], Act.Silu)
                else:
                    qsil = qb[pair] if pair < 4 else kb[pair - 4]
                    qs[pair] = qsil
                    nc.scalar.activation(qsil[:], cur[:], Act.Silu)
                    sq = smt.tile([128, T], bf16, tag="sq", name="sq", bufs=1)
                    nc.scalar.activation(sq[:], qsil[:], Act.Square)
                    for half in range(2):
                        pss = pr.tile([8, 512], fp32, tag="sqs", name="sqs")
                        nc.tensor.matmul(pss[:], oh8t[:, pair * 8:pair * 8 + 8],
                                         sq[:, half * 512:(half + 1) * 512],
                                         start=True, stop=True)
                        nc.vector.tensor_tensor(ssqsb[:, half * 512:(half + 1) * 512],
                                                ssqsb[:, half * 512:(half + 1) * 512],
                                                pss[:], op=Alu.add)

        with tc.tile_pool(name="wallAp", bufs=1) as wallAp, \
             tc.tile_pool(name="convp", bufs=2) as convp, \
             tc.tile_pool(name="smt", bufs=2) as smt, \
             tc.tile_pool(name="pps", bufs=1, space="PSUM") as pr:
            # PSUM tags: proj(2) sqs(1) bps(1) gps(1) nb(2) = 7 banks
            ssqsb = smt.tile([8, T], fp32, tag="ssqsb", name="ssqsb", bufs=1)
            nc.vector.memset(ssqsb[:], 0.0)
            wtA = []
            for k in range(16):
                dma(ht[k][:], hT[k * 128:(k + 1) * 128, :])
                wA = wallAp.tile([128, 1668], bf16, tag=f"wA{k}", name=f"wA{k}")
                dma(wA[:, 0:1536], wall[k * 128:(k + 1) * 128, 0:1536])
                dma(wA[:, 1536:1668], wall[k * 128:(k + 1) * 128, WF10:WF10 + 132])
                wtA.append(wA)
            for k in range(16):
                dma(wtB[k][:], wall[k * 128:(k + 1) * 128, WG0:WG0 + 512])
            for m in range(4):
                project(wtA, pr, 0, m, conv_slot=0, pair=m)
            for m in range(4):
                project(wtA, pr, 512, m, conv_slot=1, pair=4 + m)
            for m in range(4):
                project(wtA, pr, 1024, m, dst_bf16=vb[m], conv_slot=2)

            # f1 projection
            for half in range(2):
                ps = pr.tile([128, 512], fp32, tag="proj", name="f1ps", bufs=2)
                for k in range(16):
                    nc.tensor.matmul(ps[:], wtA[k][:, 1536:1664],
                                     ht[k][:, half * 512:(half + 1) * 512],
                                     start=(k == 0), stop=(k == 15))
                nc.scalar.copy(f1b[:, half * 512:(half + 1) * 512], ps[:])

            # beta (sigmoid) then gate-softplus chain, table-load friendly order
            for half in range(2):
                bps = pr.tile([4, 512], fp32, tag="bps", name="bps")
                for k in range(16):
                    nc.tensor.matmul(bps[:], wtA[k][:, 1664:1668],
                                     ht[k][:, half * 512:(half + 1) * 512],
                                     start=(k == 0), stop=(k == 15))
                nc.scalar.activation(bsg[:, half * 512:(half + 1) * 512], bps[:],
                                     Act.Sigmoid)
            sp1s = []
            for half in range(2):
                for h in range(4):
                    gps = pr.tile([8, 512], fp32, tag="gps", name="gps", bufs=2)
                    nc.tensor.matmul(gps[:], wf2t[:, h * 8:(h + 1) * 8],
                                     f1b[:, half * 512:(half + 1) * 512],
                                     start=True, stop=True)
                    spe = smt.tile([8, 512], bf16, tag="spe", name="spe", bufs=2)
                    nc.scalar.activation(spe[:], gps[:], Act.Exp,
                                         bias=dtbt[:, h:h + 1])
                    sp1 = smt.tile([8, 512], bf16, tag="sp1", name="sp1", bufs=8)
                    nc.vector.tensor_scalar(sp1[:], spe[:], 1.0, None, op0=Alu.add)
                    sp1s.append((half, h, sp1))
            # all Ln together: l2 normalizer + softplus logs
            nrm = smt.tile([8, T], fp32, tag="nrm", name="nrm", bufs=1)
            nc.scalar.activation(nrm[:], ssqsb[:], Act.Ln, scale=sc8t[:, 0:1],
                                 bias=eps8t[:, 0:1])
            for half, h, sp1 in sp1s:
                sp = smt.tile([8, 512], bf16, tag="sp", name="sp", bufs=2)
                nc.scalar.activation(sp[:], sp1[:], Act.Ln)
                nc.vector.tensor_scalar(gna8[h][:, half * 512:(half + 1) * 512],
                                        sp[:], negat8(h), None, op0=Alu.mult)
            recb = smt.tile([8, T], bf16, tag="recb", name="recb", bufs=1)
            nc.scalar.activation(recb[:], nrm[:], Act.Exp, scale=-0.5)
            for pair in range(8):
                dst = qb[pair] if pair < 4 else kb[pair - 4]
                for half in range(2):
                    nb = pr.tile([128, 512], fp32, tag="nb", name="nb", bufs=2)
                    nc.tensor.matmul(nb[:], s8b[:, pair * 128:(pair + 1) * 128],
                                     recb[:, half * 512:(half + 1) * 512],
                                     start=True, stop=True)
                    nc.vector.tensor_tensor(dst[:, half * 512:(half + 1) * 512],
                                            qs[pair][:, half * 512:(half + 1) * 512],
                                            nb[:], op=Alu.mult)

        pers2 = ctx.enter_context(tc.tile_pool(name="pers2", bufs=1))
        yb = [pers2.tile([128, T], bf16, tag=f"yb{m}", name=f"yb{m}")
              for m in range(4)]
        wotp = ctx.enter_context(tc.tile_pool(name="wotp", bufs=1))
        wot = [wotp.tile([128, D], bf16, tag=f"wo{k}", name=f"wo{k}") for k in range(4)]

        # ================= recurrence =================
        rctx = ExitStack()
        rc = rctx.enter_context(tc.tile_pool(name="rc", bufs=2))
        rr = rctx.enter_context(tc.tile_pool(name="rr", bufs=3))
        prc = rctx.enter_context(tc.tile_pool(name="prc", bufs=1, space="PSUM"))
        # PSUM tags: tp(2) big(2) dblx(2) mx(2) = 8 banks

        hdt = lambda nm, h, sh, dt=bf16, bufs=2: rc.tile(
            sh, dt, tag=f"{nm}{h}", name=f"{nm}{h}", bufs=bufs)

        MSKN = {1: 0, 3: 1, 5: 2, 6: 3, 7: 4}

        def pro_vt(P, early=False):
            ts = P['ts']
            P['vt'], P['kts'] = [], []
            for h in range(4):
                vtp = prc.tile([128, C], bf16, tag="tp", name="vtp", bufs=2)
                nc.tensor.transpose(vtp[:], vb[h][:, ts], idb[:])
                vt = hdt("vt", h, [128, C])
                if early:
                    nc.vector.tensor_copy(vt[:], vtp[:])
                else:
                    nc.scalar.copy(vt[:], vtp[:])
                P['vt'].append(vt)
                ktp = prc.tile([128, C], bf16, tag="tp", name="ktp", bufs=2)
                nc.tensor.transpose(ktp[:], P['_kends'][h][:], idb[:])
                kts = hdt("kts", h, [128, C])
                nc.vector.tensor_scalar(kts[:], ktp[:], P['_beta2'][:, h:h + 1],
                                        None, op0=Alu.mult)
                P['kts'].append(kts)

        def prologue(ci, with_vt=True, early=False):
            ts = slice(ci * C, (ci + 1) * C)
            P = {'ts': ts}
            cn8s = []
            for h in range(4):
                cn8 = hdt("cn8", h, [8, C], fp32)
                nc.vector.tensor_tensor_scan(cn8[:], ones32[0:8, :],
                                             gna8[h][:, ts], 0.0,
                                             op0=Alu.mult, op1=Alu.add)
                cn8s.append(cn8)
            cnt8s = []
            for h in range(4):
                cNtp = prc.tile([128, 8], fp32, tag="tp", name="cNtp", bufs=2)
                nc.tensor.transpose(cNtp[:], cn8s[h][:], idf[0:8, 0:8])
                cnt8 = hdt("cnt8", h, [128, 8], fp32)
                nc.scalar.copy(cnt8[:], cNtp[:])
                cnt8s.append(cnt8)
            b2p = prc.tile([128, 4], fp32, tag="tp", name="b2p", bufs=2)
            nc.tensor.transpose(b2p[:], bsg[:, ts], idf[0:4, 0:4])
            beta2 = rc.tile([128, 4], fp32, tag="beta2", name="beta2")
            nc.scalar.copy(beta2[:], b2p[:])
            kmsks = []
            for h in range(4):
                kmsk = rr.tile([128, 5 * C], bf16, tag="kmsk", name="kmsk")
                for n, j in MSKN.items():
                    dst = kmsk[:, j * C:(j + 1) * C]
                    if j < 3:
                        nc.scalar.mul(dst, kb[h][:, ts], gmct[:, n:n + 1])
                    else:
                        nc.vector.tensor_scalar(dst, kb[h][:, ts], gmct[:, n:n + 1],
                                                None, op0=Alu.mult)
                kmsks.append(kmsk)
            exp8s, exp8ks = [], []
            for h in range(4):
                e8 = hdt("exp8", h, [8, C], fp32)
                nc.scalar.activation(e8[:], cn8s[h][:], Act.Exp)
                exp8s.append(e8)
            for h in range(4):
                e8k = hdt("exp8k", h, [8, C], fp32)
                nc.scalar.activation(e8k[:], cn8s[h][:], Act.Exp, scale=-1.0,
                                     bias=cn8s[h][:, C - 1:C])
                exp8ks.append(e8k)
            bfks = []
            for h in range(4):
                bfk = prc.tile([128, 256], fp32, tag="tp", name="bfk", bufs=2)
                nc.tensor.matmul(bfk[:, 0:128], replt, exp8s[h][:],
                                 start=True, stop=True)
                nc.tensor.matmul(bfk[:, 128:256], replt, exp8ks[h][:],
                                 start=True, stop=True)
                bfks.append(bfk)
            P['bC'] = []
            for h in range(4):
                bC = hdt("bC", h, [128, 1], fp32)
                nc.scalar.copy(bC[:], bfks[h][:, 127:128])
                P['bC'].append(bC)
            P['Wt'], P['qtT'], kends = [], [], []
            for h in range(4):
                Wth = hdt("Wt", h, [128, C])
                nc.vector.tensor_tensor(Wth[:], kb[h][:, ts], bfks[h][:, 0:128],
                                        op=Alu.mult)
                P['Wt'].append(Wth)
                qtTh = hdt("qtT", h, [128, C])
                nc.vector.tensor_tensor(qtTh[:], qb[h][:, ts], bfks[h][:, 0:128],
                                        op=Alu.mult)
                P['qtT'].append(qtTh)
                kendh = hdt("kend", h, [128, C])
                nc.vector.tensor_tensor(kendh[:], kb[h][:, ts], bfks[h][:, 128:256],
                                        op=Alu.mult)
                kends.append(kendh)
            ealls = []
            for h in range(4):
                bcaL = prc.tile([128, 512], fp32, tag="big", name="bcaL", bufs=2)
                bcaH = prc.tile([128, 512], fp32, tag="big", name="bcaH", bufs=2)
                for n in range(8):
                    dst = bcaL if n < 4 else bcaH
                    nc.tensor.matmul(dst[:, (n % 4) * C:(n % 4 + 1) * C],
                                     sel8[:, n * 128:(n + 1) * 128], cn8s[h][:],
                                     start=True, stop=True)
                eallin = rr.tile([128, 8 * C], bf16, tag="eallin", name="eallin")
                nrelu = 0 if early else 4
                for n in range(nrelu):
                    nc.scalar.activation(eallin[:, n * C:(n + 1) * C],
                                         bcaL[:, n * C:(n + 1) * C],
                                         Act.Relu, scale=-1.0,
                                         bias=cnt8s[h][:, n:n + 1])
                for n in range(nrelu, 8):
                    bsrc = bcaL if n < 4 else bcaH
                    nc.vector.tensor_scalar(eallin[:, n * C:(n + 1) * C],
                                            bsrc[:, (n % 4) * C:(n % 4 + 1) * C],
                                            cnt8s[h][:, n:n + 1], 0.0,
                                            op0=Alu.subtract, op1=Alu.min)
                eall = rr.tile([128, 8 * C], bf16, tag="eall", name="eall", bufs=4)
                if nrelu:
                    nc.scalar.activation(eall[:, 0:nrelu * C], eallin[:, 0:nrelu * C],
                                         Act.Exp, scale=-1.0)
                nc.scalar.activation(eall[:, nrelu * C:], eallin[:, nrelu * C:],
                                     Act.Exp)
                ealls.append(eall)
            P['A'] = [[None] * 7 for _ in range(4)]
            P['GtM'] = []
            for h in range(4):
                pls = []
                for csrc in (kb[h], qb[h]):
                    pl = prc.tile([128, 512], fp32, tag="big", name="pall", bufs=2)
                    ph = prc.tile([128, 512], fp32, tag="big", name="pallh", bufs=2)
                    for n in range(8):
                        dst = pl if n < 4 else ph
                        if n in (0, 2, 4):
                            nc.tensor.matmul(dst[:, (n % 4) * C:(n % 4 + 1) * C],
                                             kb[h][16 * n:16 * (n + 1), ts],
                                             csrc[16 * n:16 * (n + 1), ts],
                                             start=True, stop=True)
                        else:
                            j = MSKN[n]
                            nc.tensor.matmul(dst[:, (n % 4) * C:(n % 4 + 1) * C],
                                             kmsks[h][:, j * C:(j + 1) * C],
                                             csrc[:, ts],
                                             start=True, stop=True)
                    pls.append((pl, ph))
                prods = []
                for x, (pl, ph) in enumerate(pls):
                    prod = rr.tile([128, 8 * C], bf16, tag="prod", name=f"prod{x}", bufs=4)
                    if x == 1:
                        # G-corr is off the critical chain: drain psum via Act,
                        # then cheap bf16 TTs on DVE
                        pg = rr.tile([128, 8 * C], bf16, tag="pgd", name="pgd",
                                     bufs=2)
                        nc.scalar.copy(pg[:, 0:4 * C], pl[:])
                        nc.scalar.copy(pg[:, 4 * C:], ph[:])
                        nc.vector.tensor_tensor(prod[:], ealls[h][:], pg[:],
                                                op=Alu.mult)
                    else:
                        nc.vector.tensor_tensor(prod[:, 0:4 * C], ealls[h][:, 0:4 * C],
                                                pl[:], op=Alu.mult)
                        nc.vector.tensor_tensor(prod[:, 4 * C:], ealls[h][:, 4 * C:],
                                                ph[:], op=Alu.mult)
                    prods.append(prod)
                for x, prod in enumerate(prods):
                    # sum the 8 group blocks on PE (identity-lhsT accumulation)
                    t1 = prc.tile([128, C], fp32, tag="dblx", name="t1p", bufs=2)
                    for n in range(8):
                        nc.tensor.matmul(t1[:], idb, prod[:, n * C:(n + 1) * C],
                                         start=(n == 0), stop=(n == 7))
                    if x == 0:
                        A0 = rc.tile([128, C], bf16, tag=f"A{h}", name=f"A{h}_0",
                                     bufs=12)
                        nc.vector.scalar_tensor_tensor(A0[:], t1[:],
                                                       beta2[:, h:h + 1], mMt[:],
                                                       op0=Alu.mult, op1=Alu.mult)
                        P['A'][h][0] = A0
                    else:
                        GtM = hdt("GtM", h, [128, C])
                        nc.vector.scalar_tensor_tensor(GtM[:], t1[:],
                                                       beta2[:, h:h + 1], mGt[:],
                                                       op0=Alu.mult, op1=Alu.mult)
                        P['GtM'].append(GtM)
            P['_kends'] = kends
            P['_beta2'] = beta2
            if with_vt:
                pro_vt(P)
            # A/B doubling chains (state-independent)
            Bs = [[None] * 6 for _ in range(4)]
            for h in range(4):
                b0p = prc.tile([128, C], bf16, tag="tp", name="b0p", bufs=2)
                nc.tensor.transpose(b0p[:], P['A'][h][0][:], idb[:])
                B0 = rc.tile([128, C], bf16, tag=f"B{h}", name=f"B{h}_0", bufs=2)
                nc.scalar.copy(B0[:], b0p[:])
                Bs[h][0] = B0
            for lev in range(1, 7):
                for h in range(4):
                    dbl = prc.tile([128, 256], fp32, tag="dblx", name="dbl", bufs=2)
                    nc.tensor.matmul(dbl[:, 0:128], Bs[h][lev - 1][:],
                                     P['A'][h][lev - 1][:], start=True, stop=True)
                    if lev < 6:
                        nc.tensor.matmul(dbl[:, 128:256], P['A'][h][lev - 1][:],
                                         Bs[h][lev - 1][:], start=True, stop=True)
                    An = rc.tile([128, C], bf16, tag=f"A{h}", name=f"A{h}_{lev}",
                                 bufs=12)
                    nc.scalar.copy(An[:], dbl[:, 0:128])
                    P['A'][h][lev] = An
                    if lev < 6:
                        Bn = rc.tile([128, C], bf16, tag=f"B{h}",
                                     name=f"B{h}_{lev}", bufs=2)
                        if lev % 2 == 0 and not early:
                            nc.scalar.copy(Bn[:], dbl[:, 128:256])
                        else:
                            nc.vector.tensor_copy(Bn[:], dbl[:, 128:256])
                        Bs[h][lev] = Bn
            return P

        def spine(ci, P):
            ts = P['ts']
            po, pn = ci % 2, (ci + 1) % 2
            xbs = []
            for h in range(4):
                ws0 = prc.tile([128, C], fp32, tag="tp", name="ws0", bufs=2)
                nc.tensor.matmul(ws0[:], P['Wt'][h][:], Sb[h][po][:],
                                 start=True, stop=True)
                xb = rc.tile([128, C], bf16, tag=f"xb{h}", name=f"xb{h}", bufs=3)
                nc.vector.tensor_tensor(xb[:], P['vt'][h][:], ws0[:],
                                        op=Alu.subtract)
                xbs.append(xb)
            for lev in range(7):
                for h in range(4):
                    mx = prc.tile([128, C], fp32, tag="mx", name="mx", bufs=2)
                    nc.tensor.matmul(mx[:], P['A'][h][lev][:], xbs[h][:],
                                     start=True, stop=True)
                    xn = rc.tile([128, C], bf16, tag=f"xb{h}", name=f"xb{h}_{lev}",
                                 bufs=3)
                    nc.vector.tensor_tensor(xn[:], xbs[h][:], mx[:],
                                            op=(Alu.subtract if lev == 0 else Alu.add))
                    xbs[h] = xn
            for h in range(4):
                sup = prc.tile([128, DV], fp32, tag="tp", name="sup", bufs=2)
                nc.tensor.matmul(sup[:], P['kts'][h][:], xbs[h][:],
                                 start=True, stop=True)
                nc.vector.scalar_tensor_tensor(Sf[h][pn][:], Sf[h][po][:],
                                               P['bC'][h][:, 0:1], sup[:],
                                               op0=Alu.mult, op1=Alu.add)
                nc.vector.scalar_tensor_tensor(Sb[h][pn][:], Sf[h][po][:],
                                               P['bC'][h][:, 0:1], sup[:],
                                               op0=Alu.mult, op1=Alu.add)
                otp = prc.tile([128, C], fp32, tag="tp", name="otp", bufs=2)
                nc.tensor.matmul(otp[:], Sb[h][po][:], P['qtT'][h][:],
                                 start=True, stop=False)
                nc.tensor.matmul(otp[:], xbs[h][:], P['GtM'][h][:],
                                 start=False, stop=True)
                nc.vector.tensor_tensor(yb[h][:, ts], gateb[h][:, ts], otp[:],
                                        op=Alu.mult)

        pros = [None, None]
        pros[0] = prologue(0, with_vt=False, early=True)
        pros[1] = prologue(1, with_vt=False, early=True)
        # g projection emitted here: its PE work overlaps prologue 0/1
        for m in range(4):
            project(wtB, prc, 0, m, dst_bf16=gateb[m],
                    gate_bias=bgt[:, m:m + 1], ptag="mx", pbufs=2)
        for k in range(4):
            dma(wot[k][:], wo[k * 128:(k + 1) * 128, :])
        pro_vt(pros[0], early=True)
        pro_vt(pros[1], early=True)
        spine(0, pros[0])
        for ci in range(2, NCH):
            pros[ci % 2] = prologue(ci)
            spine(ci - 1, pros[(ci - 1) % 2])
        spine(NCH - 1, pros[(NCH - 1) % 2])

        rctx.close()
        # ================= deferred RMSNorm + output projection =================
        with tc.tile_pool(name="post", bufs=2) as post, \
             tc.tile_pool(name="ppc", bufs=2, space="PSUM") as ppc:
            # PSUM tags: ssp(2) rbc(2) proj(2) = 6 banks
            ysqs = []
            for h in range(4):
                ysq = post.tile([128, T], bf16, tag="ysq", name="ysq", bufs=4)
                nc.scalar.activation(ysq[:], yb[h][:], Act.Square)
                ysqs.append(ysq)
            nrcs = []
            for h in range(4):
                nrc = post.tile([1, T], fp32, tag="nrc", name="nrc", bufs=4)
                for half in range(2):
                    ssp = ppc.tile([1, 512], fp32, tag="ssp", name="ssp")
                    nc.tensor.matmul(ssp[:], octb[:],
                                     ysqs[h][:, half * 512:(half + 1) * 512],
                                     start=True, stop=True)
                    nc.scalar.activation(nrc[:, half * 512:(half + 1) * 512],
                                         ssp[:], Act.Ln, scale=1.0 / DV,
                                         bias=epsnt[:, 0:1])
                nrcs.append(nrc)
            for h in range(4):
                rcb = post.tile([1, T], bf16, tag="rcb", name="rcb", bufs=4)
                nc.scalar.activation(rcb[:], nrcs[h][:], Act.Exp, scale=-0.5)
                for half in range(2):
                    rbc = ppc.tile([128, 512], fp32, tag="rbc", name="rbc")
                    nc.tensor.matmul(rbc[:], o1b[:], rcb[:, half * 512:(half + 1) * 512],
                                     start=True, stop=True)
                    nc.vector.scalar_tensor_tensor(yb[h][:, half * 512:(half + 1) * 512],
                                                   yb[h][:, half * 512:(half + 1) * 512],
                                                   nwt[:, 0:1], rbc[:],
                                                   op0=Alu.mult, op1=Alu.mult)
            # output projection
            for m in range(16):
                osb = post.tile([128, T], fp32, tag="osb", name="osb")
                for half in range(2):
                    ps = ppc.tile([128, 512], fp32, tag="proj", name="ops")
                    for k in range(4):
                        nc.tensor.matmul(ps[:], wot[k][:, m * 128:(m + 1) * 128],
                                         yb[k][:, half * 512:(half + 1) * 512],
                                         start=(k == 0), stop=(k == 3))
                    if half == 0:
                        nc.vector.tensor_copy(osb[:, 0:512], ps[:])
                    else:
                        nc.scalar.copy(osb[:, 512:1024], ps[:])
                dma(outT[m * 128:(m + 1) * 128, :], osb[:])

    nc.compile()
    return nc


def _prep_inputs(inputs):
    f32 = np.float32
    hs = np.asarray(inputs['hidden_states'], f32)
    tri = np.tril(np.ones((C, C), f32))
    maskM = (1.0 - tri).astype(f32)
    maskG = (1.0 - tri + np.eye(C, dtype=f32)).astype(f32)
    repl = np.zeros((NG, DK), f32)
    for n in range(NG):
        repl[n, n * GG:(n + 1) * GG] = 1.0
    sel8 = np.zeros((NG, NG * 128), f32)
    for n in range(NG):
        sel8[n, n * 128:(n + 1) * 128] = 1.0
    oh8 = np.zeros((DK, 64), f32)
    for i in range(8):
        oh8[:, i * 8 + i] = 1.0
    ident = np.eye(128, dtype=f32)

    maps = []
    for c in range(8):
        b, hg = c // 4, c % 4
        cols = slice(hg * NH * DK, (hg + 1) * NH * DK)
        gcols = slice(hg * NH * NG, (hg + 1) * NH * NG)
        hcols = slice(hg * NH, (hg + 1) * NH)
        nega = -np.exp(np.repeat(np.asarray(inputs['A_log'], f32)[hcols], NG))

        packf = np.zeros((128, NF), f32)
        cw = np.concatenate(
            [np.asarray(inputs['conv_q'], f32)[cols],
             np.asarray(inputs['conv_k'], f32)[cols],
             np.asarray(inputs['conv_v'], f32)[cols]], 1)  # [512, 12]
        for m in range(4):
            packf[:, PF_CW + m * 12:PF_CW + (m + 1) * 12] = cw[m * 128:(m + 1) * 128]
        packf[:, PF_BG:PF_BG + 4] = np.asarray(inputs['bg'], f32)[cols].reshape(NH, DV).T
        packf[:, PF_NW] = np.asarray(inputs['norm_w'], f32)
        packf[0:8, PF_NEGA:PF_NEGA + 4] = nega.reshape(NH, NG).T
        packf[0:8, PF_DTB:PF_DTB + 4] = (
            np.asarray(inputs['dt_bias'], f32)[gcols].reshape(NH, NG).T)
        packf[0:8, PF_SC8] = [1.0 / SCALE ** 2] * 4 + [1.0] * 4
        packf[0:8, PF_EPS8] = [1e-6 / SCALE ** 2] * 4 + [1e-6] * 4
        packf[0:1, PF_EPSN] = EPS
        packf[0:8, PF_REPL:PF_REPL + 128] = repl
        packf[:, PF_IDF:PF_IDF + 128] = ident
        packf[0:8, PF_SEL:PF_SEL + 1024] = sel8
        packf[:, PF_GMC:PF_GMC + 8] = repl.T

        packb = np.zeros((128, NB), f32)
        packb[:, PB_OH8:PB_OH8 + 64] = oh8
        packb[0:8, PB_S8B:PB_S8B + 1024] = sel8
        packb[:, PB_MM:PB_MM + 128] = maskM
        packb[:, PB_MG:PB_MG + 128] = maskG
        packb[:, PB_IDB:PB_IDB + 128] = ident
        packb[:, PB_OCT] = 1.0
        packb[0:1, PB_O1B:PB_O1B + 128] = 1.0

        wallm = np.concatenate(
            [np.asarray(inputs['Wq'], f32)[:, cols],
             np.asarray(inputs['Wk'], f32)[:, cols],
             np.asarray(inputs['Wv'], f32)[:, cols],
             np.asarray(inputs['Wg'], f32)[:, cols],
             np.asarray(inputs['Wf1'], f32),
             np.asarray(inputs['Wb'], f32)[:, hcols]], 1)

        m = {
            'hT': np.ascontiguousarray(hs[b].T).astype(BF),
            'wall': np.ascontiguousarray(wallm).astype(BF),
            'wo': np.ascontiguousarray(np.asarray(inputs['Wo'], f32)[cols, :]).astype(BF),
            'wf2': np.ascontiguousarray(np.asarray(inputs['Wf2'], f32)[:, gcols]).astype(BF),
            'packf': packf,
            'packb': packb.astype(BF),
        }
        maps.append(m)
    return maps


def kernel(**inputs):
    from concourse.bass_utils import run_bass_kernel_spmd
    if 'nc' not in _CACHE:
        _CACHE['nc'] = _build()
    nc = _CACHE['nc']
    maps = _prep_inputs(inputs)
    res = run_bass_kernel_spmd(nc, maps, list(range(8))).results
    out = np.zeros((B, T, D), np.float32)
    for c in range(8):
        out[c // 4] += res[c]['outT'].T.astype(np.float32)
    return out


# revision 48
# speedup vs baseline: 1.0886x; 1.0446x over previous
"""Grouped gated DeltaNet (KDA-style) on 8 TRN2 NeuronCores.

Sharding: core c -> (batch b = c//4, head-group hg = c%4 of 4 heads).
Per core: column-sharded projections (weights resident, loaded once),
short-conv+silu, l2norm, chunked gated delta-rule recurrence (chunk
C=128, group decay via 1-partition f32r broadcast matmuls + fused
sub/clamp, 16-partition group correlation matmuls, transpose-free A/B
doubling with interleaved triangular-solve application), deferred gated
RMSNorm (batched over T), row-shard output projection. Host sums 4
partials per batch.

Self-contained: B=2, T=1024, D=2048, H=16, DK=DV=128 hardcoded.
"""
import sys
sys.path.insert(0, '/opt/trn_rl_repo')
import numpy as np
import ml_dtypes
from contextlib import ExitStack

B, T, D = 2, 1024, 2048
H, DK, DV, GG = 16, 128, 128, 16
NG = DK // GG          # 8 gate groups per head
NH = 4                 # heads per core
C = 128                # chunk length
NCH = T // C
SCALE = DK ** -0.5
EPS = 1e-5

# packf fp32 column offsets
PF_CW = 0        # 4 blocks x 12
PF_BG = 48
PF_NW = 52
PF_NEGA = 53     # [8,4] (n,h)
PF_DTB = 57      # [8,4]
PF_SC8 = 61
PF_EPS8 = 62
PF_EPSN = 63
PF_REPL = 64     # [8,128]
PF_IDF = 192     # [128,128]
PF_SEL = 320     # [8,1024] group-selector
PF_GMC = 1344    # [128,8] group row-mask cols
NF = 1352
# packb bf16 column offsets
PB_OH8 = 0       # [128,64]
PB_S8B = 64      # [8,1024]
PB_MM = 1088     # [128,128]
PB_MG = 1216
PB_IDB = 1344
PB_OCT = 1472    # [128,1]
PB_O1B = 1473    # [1,128]
NB = 1601

WQ0, WK0, WV0, WG0, WF10, WB0 = 0, 512, 1024, 1536, 2048, 2176
WALLC = 2180

BF = ml_dtypes.bfloat16
_CACHE = {}

FP32_CHAIN = False   # fp32 x-chain fallback (precision)


def _build():
    import concourse.tile as tile
    from concourse import bacc, mybir

    fp32 = mybir.dt.float32
    f32r = mybir.dt.float32r
    bf16 = mybir.dt.bfloat16
    Alu = mybir.AluOpType
    Act = mybir.ActivationFunctionType

    nc = bacc.Bacc("TRN2", target_bir_lowering=False, debug=False, num_devices=8)
    dp = lambda n, sh, dt: nc.dram_tensor(n, sh, dt, kind="ExternalInput").ap()
    hT = dp("hT", [D, T], bf16)
    wall = dp("wall", [D, WALLC], bf16)
    wo = dp("wo", [NH * DV, D], bf16)
    wf2 = dp("wf2", [DV, NH * NG], bf16)
    packf = dp("packf", [128, NF], fp32)
    packb = dp("packb", [128, NB], bf16)
    outT = nc.dram_tensor("outT", [D, T], fp32, kind="ExternalOutput").ap()

    with tile.TileContext(nc) as tc, ExitStack() as ctx:
        pool = lambda name, bufs, space="SBUF": ctx.enter_context(
            tc.tile_pool(name=name, bufs=bufs, space=space))

        cons = pool("cons", 1)
        pers = pool("pers", 1)
        stp = pool("st", 1)

        dma = nc.sync.dma_start

        pf = cons.tile([128, NF], fp32, tag="packf", name="packf")
        dma(pf[:], packf[:])
        pb = cons.tile([128, NB], bf16, tag="packb", name="packb")
        dma(pb[:], packb[:])
        wf2t = cons.tile([128, 32], bf16, tag="wf2t", name="wf2t")
        dma(wf2t[:], wf2[:])

        cwt = lambda m: pf[:, PF_CW + m * 12: PF_CW + (m + 1) * 12]
        bgt = pf[:, PF_BG:PF_BG + 4]
        nwt = pf[:, PF_NW:PF_NW + 1]
        negat8 = lambda h: pf[0:8, PF_NEGA + h:PF_NEGA + h + 1]
        dtbt = pf[0:8, PF_DTB:PF_DTB + 4]
        sc8t = pf[0:8, PF_SC8:PF_SC8 + 1]
        eps8t = pf[0:8, PF_EPS8:PF_EPS8 + 1]
        epsnt = pf[0:1, PF_EPSN:PF_EPSN + 1]
        replt = pf[0:8, PF_REPL:PF_REPL + 128]
        idf = pf[:, PF_IDF:PF_IDF + 128]
        sel8 = pf[0:8, PF_SEL:PF_SEL + 1024]
        oh8t = pb[:, PB_OH8:PB_OH8 + 64]
        s8b = pb[0:8, PB_S8B:PB_S8B + 1024]
        mMt = pb[:, PB_MM:PB_MM + 128]
        mGt = pb[:, PB_MG:PB_MG + 128]
        idb = pb[:, PB_IDB:PB_IDB + 128]
        octb = pb[:, PB_OCT:PB_OCT + 1]
        o1b = pb[0:1, PB_O1B:PB_O1B + 128]
        gmct = pf[:, PF_GMC:PF_GMC + 8]

        ones32 = cons.tile([32, C], fp32, tag="ones32", name="ones32")
        nc.vector.memset(ones32[:], 1.0)

        # ---- persistent activations ----
        mk = lambda nm: [pers.tile([128, T], bf16, tag=f"{nm}{m}", name=f"{nm}{m}")
                         for m in range(4)]
        qb, kb, vb = mk("qb"), mk("kb"), mk("vb")
        gateb = mk("gateb")
        f1b = pers.tile([128, T], bf16, tag="f1b", name="f1b")
        gna8 = [pers.tile([8, T], bf16, tag=f"gna{h}", name=f"gna{h}")
                for h in range(4)]
        bsg = pers.tile([4, T], fp32, tag="bsg", name="bsg")

        # ---- state tiles (parity pairs) ----
        Sf = [[stp.tile([128, DV], fp32, tag=f"Sf{h}_{p}", name=f"Sf{h}_{p}")
               for p in range(2)] for h in range(4)]
        Sb = [[stp.tile([128, DV], bf16, tag=f"Sb{h}_{p}", name=f"Sb{h}_{p}")
               for p in range(2)] for h in range(4)]
        for h in range(4):
            nc.vector.memset(Sf[h][0][:], 0.0)
            nc.vector.memset(Sb[h][0][:], 0.0)

        # ================= projections =================
        htp = ctx.enter_context(tc.tile_pool(name="htp", bufs=1))
        wallBp = ctx.enter_context(tc.tile_pool(name="wallBp", bufs=1))
        ht = [htp.tile([128, T], bf16, tag=f"ht{k}", name=f"ht{k}")
              for k in range(16)]
        wtB = [wallBp.tile([128, 512], bf16, tag=f"wB{k}", name=f"wB{k}")
               for k in range(16)]
        qs = {}

        def project(wts, pr, col0, m, dst_bf16=None, conv_slot=None, pair=None,
                    gate_bias=None, ptag="proj", pbufs=2):
                xpad = None
                if conv_slot is not None:
                    xpad = convp.tile([128, T + 3], fp32, tag="xpad", name="xpad")
                    nc.vector.memset(xpad[:, 0:3], 0.0)
                for half in range(2):
                    ps = pr.tile([128, 512], fp32, tag=ptag, name="projps", bufs=pbufs)
                    for k in range(16):
                        nc.tensor.matmul(ps[:], wts[k][:, col0 + m * 128:col0 + (m + 1) * 128],
                                         ht[k][:, half * 512:(half + 1) * 512],
                                         start=(k == 0), stop=(k == 15))
                    if xpad is not None:
                        nc.scalar.copy(xpad[:, 3 + half * 512: 3 + (half + 1) * 512], ps[:])
                    elif gate_bias is not None:
                        nc.scalar.activation(dst_bf16[:, half * 512:(half + 1) * 512],
                                             ps[:], Act.Silu, bias=gate_bias)
                    else:
                        nc.scalar.copy(dst_bf16[:, half * 512:(half + 1) * 512], ps[:])
                if xpad is None:
                    return
                cwm = cwt(m)
                s = conv_slot * 4
                a = convp.tile([128, T], fp32, tag="acca", name="acca", bufs=1)
                bt = convp.tile([128, T], fp32, tag="accb", name="accb", bufs=1)
                nc.vector.tensor_scalar(a[:], xpad[:, 3:3 + T], cwm[:, s + 3:s + 4],
                                        None, op0=Alu.mult)
                cur, nxt = a, bt
                for kk in (2, 1, 0):
                    nc.vector.scalar_tensor_tensor(nxt[:], xpad[:, kk:kk + T],
                                                   cwm[:, s + kk:s + kk + 1], cur[:],
                                                   op0=Alu.mult, op1=Alu.add)
                    cur, nxt = nxt, cur
                if pair is None:
                    nc.scalar.activation(dst_bf16[:], cur[:], Act.Silu)
                else:
                    qsil = qb[pair] if pair < 4 else kb[pair - 4]
                    qs[pair] = qsil
                    nc.scalar.activation(qsil[:], cur[:], Act.Silu)
                    sq = smt.tile([128, T], bf16, tag="sq", name="sq", bufs=1)
                    nc.scalar.activation(sq[:], qsil[:], Act.Square)
                    for half in range(2):
                        pss = pr.tile([8, 512], fp32, tag="sqs", name="sqs")
                        nc.tensor.matmul(pss[:], oh8t[:, pair * 8:pair * 8 + 8],
                                         sq[:, half * 512:(half + 1) * 512],
                                         start=True, stop=True)
                        nc.vector.tensor_tensor(ssqsb[:, half * 512:(half + 1) * 512],
                                                ssqsb[:, half * 512:(half + 1) * 512],
                                                pss[:], op=Alu.add)

        with tc.tile_pool(name="wallAp", bufs=1) as wallAp, \
             tc.tile_pool(name="convp", bufs=2) as convp, \
             tc.tile_pool(name="smt", bufs=2) as smt, \
             tc.tile_pool(name="pps", bufs=1, space="PSUM") as pr:
            # PSUM tags: proj(2) sqs(1) bps(1) gps(1) nb(2) = 7 banks
            ssqsb = smt.tile([8, T], fp32, tag="ssqsb", name="ssqsb", bufs=1)
            nc.vector.memset(ssqsb[:], 0.0)
            wtA = []
            for k in range(16):
                dma(ht[k][:], hT[k * 128:(k + 1) * 128, :])
                wA = wallAp.tile([128, 1668], bf16, tag=f"wA{k}", name=f"wA{k}")
                dma(wA[:, 0:1536], wall[k * 128:(k + 1) * 128, 0:1536])
                dma(wA[:, 1536:1668], wall[k * 128:(k + 1) * 128, WF10:WF10 + 132])
                wtA.append(wA)
            for k in range(16):
                dma(wtB[k][:], wall[k * 128:(k + 1) * 128, WG0:WG0 + 512])
            for m in range(4):
                project(wtA, pr, 0, m, conv_slot=0, pair=m)
            for m in range(4):
                project(wtA, pr, 512, m, conv_slot=1, pair=4 + m)
            for m in range(4):
                project(wtA, pr, 1024, m, dst_bf16=vb[m], conv_slot=2)

            # f1 projection
            for half in range(2):
                ps = pr.tile([128, 512], fp32, tag="proj", name="f1ps", bufs=2)
                for k in range(16):
                    nc.tensor.matmul(ps[:], wtA[k][:, 1536:1664],
                                     ht[k][:, half * 512:(half + 1) * 512],
                                     start=(k == 0), stop=(k == 15))
                nc.scalar.copy(f1b[:, half * 512:(half + 1) * 512], ps[:])

            # beta (sigmoid) then gate-softplus chain, table-load friendly order
            for half in range(2):
                bps = pr.tile([4, 512], fp32, tag="bps", name="bps")
                for k in range(16):
                    nc.tensor.matmul(bps[:], wtA[k][:, 1664:1668],
                                     ht[k][:, half * 512:(half + 1) * 512],
                                     start=(k == 0), stop=(k == 15))
                nc.scalar.activation(bsg[:, half * 512:(half + 1) * 512], bps[:],
                                     Act.Sigmoid)
            sp1s = []
            for half in range(2):
                for h in range(4):
                    gps = pr.tile([8, 512], fp32, tag="gps", name="gps", bufs=2)
                    nc.tensor.matmul(gps[:], wf2t[:, h * 8:(h + 1) * 8],
                                     f1b[:, half * 512:(half + 1) * 512],
                                     start=True, stop=True)
                    spe = smt.tile([8, 512], bf16, tag="spe", name="spe", bufs=2)
                    nc.scalar.activation(spe[:], gps[:], Act.Exp,
                                         bias=dtbt[:, h:h + 1])
                    sp1 = smt.tile([8, 512], bf16, tag="sp1", name="sp1", bufs=8)
                    nc.vector.tensor_scalar(sp1[:], spe[:], 1.0, None, op0=Alu.add)
                    sp1s.append((half, h, sp1))
            # all Ln together: l2 normalizer + softplus logs
            nrm = smt.tile([8, T], fp32, tag="nrm", name="nrm", bufs=1)
            nc.scalar.activation(nrm[:], ssqsb[:], Act.Ln, scale=sc8t[:, 0:1],
                                 bias=eps8t[:, 0:1])
            for half, h, sp1 in sp1s:
                sp = smt.tile([8, 512], bf16, tag="sp", name="sp", bufs=2)
                nc.scalar.activation(sp[:], sp1[:], Act.Ln)
                nc.vector.tensor_scalar(gna8[h][:, half * 512:(half + 1) * 512],
                                        sp[:], negat8(h), None, op0=Alu.mult)
            recb = smt.tile([8, T], bf16, tag="recb", name="recb", bufs=1)
            nc.scalar.activation(recb[:], nrm[:], Act.Exp, scale=-0.5)
            for pair in range(8):
                dst = qb[pair] if pair < 4 else kb[pair - 4]
                for half in range(2):
                    nb = pr.tile([128, 512], fp32, tag="nb", name="nb", bufs=2)
                    nc.tensor.matmul(nb[:], s8b[:, pair * 128:(pair + 1) * 128],
                                     recb[:, half * 512:(half + 1) * 512],
                                     start=True, stop=True)
                    nc.vector.tensor_tensor(dst[:, half * 512:(half + 1) * 512],
                                            qs[pair][:, half * 512:(half + 1) * 512],
                                            nb[:], op=Alu.mult)

        pers2 = ctx.enter_context(tc.tile_pool(name="pers2", bufs=1))
        yb = [pers2.tile([128, T], bf16, tag=f"yb{m}", name=f"yb{m}")
              for m in range(4)]
        wotp = ctx.enter_context(tc.tile_pool(name="wotp", bufs=1))
        wot = [wotp.tile([128, D], bf16, tag=f"wo{k}", name=f"wo{k}") for k in range(4)]

        # ================= recurrence =================
        rctx = ExitStack()
        rc = rctx.enter_context(tc.tile_pool(name="rc", bufs=2))
        rr = rctx.enter_context(tc.tile_pool(name="rr", bufs=3))
        prc = rctx.enter_context(tc.tile_pool(name="prc", bufs=1, space="PSUM"))
        # PSUM tags: tp(2) big(2) dblx(2) mx(2) = 8 banks

        hdt = lambda nm, h, sh, dt=bf16, bufs=2: rc.tile(
            sh, dt, tag=f"{nm}{h}", name=f"{nm}{h}", bufs=bufs)

        MSKN = {1: 0, 3: 1, 5: 2, 6: 3, 7: 4}

        def pro_vt(P, early=False):
            ts = P['ts']
            P['vt'], P['kts'] = [], []
            for h in range(4):
                vtp = prc.tile([128, C], bf16, tag="tp", name="vtp", bufs=2)
                nc.tensor.transpose(vtp[:], vb[h][:, ts], idb[:])
                vt = hdt("vt", h, [128, C])
                if early:
                    nc.vector.tensor_copy(vt[:], vtp[:])
                else:
                    nc.scalar.copy(vt[:], vtp[:])
                P['vt'].append(vt)
                ktp = prc.tile([128, C], bf16, tag="tp", name="ktp", bufs=2)
                nc.tensor.transpose(ktp[:], P['_kends'][h][:], idb[:])
                kts = hdt("kts", h, [128, C])
                nc.vector.tensor_scalar(kts[:], ktp[:], P['_beta2'][:, h:h + 1],
                                        None, op0=Alu.mult)
                P['kts'].append(kts)

        def prologue(ci, with_vt=True, early=False):
            ts = slice(ci * C, (ci + 1) * C)
            P = {'ts': ts}
            cn8s = []
            for h in range(4):
                cn8 = hdt("cn8", h, [8, C], fp32)
                nc.vector.tensor_tensor_scan(cn8[:], ones32[0:8, :],
                                             gna8[h][:, ts], 0.0,
                                             op0=Alu.mult, op1=Alu.add)
                cn8s.append(cn8)
            cnt8s = []
            for h in range(4):
                cNtp = prc.tile([128, 8], fp32, tag="tp", name="cNtp", bufs=2)
                nc.tensor.transpose(cNtp[:], cn8s[h][:], idf[0:8, 0:8])
                cnt8 = hdt("cnt8", h, [128, 8], fp32)
                nc.scalar.copy(cnt8[:], cNtp[:])
                cnt8s.append(cnt8)
            b2p = prc.tile([128, 4], fp32, tag="tp", name="b2p", bufs=2)
            nc.tensor.transpose(b2p[:], bsg[:, ts], idf[0:4, 0:4])
            beta2 = rc.tile([128, 4], fp32, tag="beta2", name="beta2")
            nc.scalar.copy(beta2[:], b2p[:])
            kmsks = []
            for h in range(4):
                kmsk = rr.tile([128, 5 * C], bf16, tag="kmsk", name="kmsk", bufs=4)
                for n, j in MSKN.items():
                    dst = kmsk[:, j * C:(j + 1) * C]
                    if j < 3:
                        nc.scalar.mul(dst, kb[h][:, ts], gmct[:, n:n + 1])
                    else:
                        nc.vector.tensor_scalar(dst, kb[h][:, ts], gmct[:, n:n + 1],
                                                None, op0=Alu.mult)
                kmsks.append(kmsk)
            exp8s, exp8ks = [], []
            for h in range(4):
                e8 = hdt("exp8", h, [8, C], fp32)
                nc.scalar.activation(e8[:], cn8s[h][:], Act.Exp)
                exp8s.append(e8)
            for h in range(4):
                e8k = hdt("exp8k", h, [8, C], fp32)
                nc.scalar.activation(e8k[:], cn8s[h][:], Act.Exp, scale=-1.0,
                                     bias=cn8s[h][:, C - 1:C])
                exp8ks.append(e8k)
            bfks = []
            for h in range(4):
                bfk = prc.tile([128, 256], fp32, tag="tp", name="bfk", bufs=2)
                nc.tensor.matmul(bfk[:, 0:128], replt, exp8s[h][:],
                                 start=True, stop=True)
                nc.tensor.matmul(bfk[:, 128:256], replt, exp8ks[h][:],
                                 start=True, stop=True)
                bfks.append(bfk)
            P['bC'] = []
            for h in range(4):
                bC = hdt("bC", h, [128, 1], fp32)
                nc.scalar.copy(bC[:], bfks[h][:, 127:128])
                P['bC'].append(bC)
            P['Wt'], P['qtT'], kends = [], [], []
            for h in range(4):
                Wth = hdt("Wt", h, [128, C])
                nc.vector.tensor_tensor(Wth[:], kb[h][:, ts], bfks[h][:, 0:128],
                                        op=Alu.mult)
                P['Wt'].append(Wth)
                qtTh = hdt("qtT", h, [128, C])
                nc.vector.tensor_tensor(qtTh[:], qb[h][:, ts], bfks[h][:, 0:128],
                                        op=Alu.mult)
                P['qtT'].append(qtTh)
                kendh = hdt("kend", h, [128, C])
                nc.vector.tensor_tensor(kendh[:], kb[h][:, ts], bfks[h][:, 128:256],
                                        op=Alu.mult)
                kends.append(kendh)
            ealls = []
            for h in range(4):
                bcaL = prc.tile([128, 512], fp32, tag="big", name="bcaL", bufs=2)
                bcaH = prc.tile([128, 512], fp32, tag="big", name="bcaH", bufs=2)
                for n in range(8):
                    dst = bcaL if n < 4 else bcaH
                    nc.tensor.matmul(dst[:, (n % 4) * C:(n % 4 + 1) * C],
                                     sel8[:, n * 128:(n + 1) * 128], cn8s[h][:],
                                     start=True, stop=True)
                eallin = rr.tile([128, 8 * C], bf16, tag="eallin", name="eallin", bufs=4)
                nrelu = 0 if early else 4
                for n in range(nrelu):
                    nc.scalar.activation(eallin[:, n * C:(n + 1) * C],
                                         bcaL[:, n * C:(n + 1) * C],
                                         Act.Relu, scale=-1.0,
                                         bias=cnt8s[h][:, n:n + 1])
                for n in range(nrelu, 8):
                    bsrc = bcaL if n < 4 else bcaH
                    nc.vector.tensor_scalar(eallin[:, n * C:(n + 1) * C],
                                            bsrc[:, (n % 4) * C:(n % 4 + 1) * C],
                                            cnt8s[h][:, n:n + 1], 0.0,
                                            op0=Alu.subtract, op1=Alu.min)
                eall = rr.tile([128, 8 * C], bf16, tag="eall", name="eall", bufs=4)
                if nrelu:
                    nc.scalar.activation(eall[:, 0:nrelu * C], eallin[:, 0:nrelu * C],
                                         Act.Exp, scale=-1.0)
                nc.scalar.activation(eall[:, nrelu * C:], eallin[:, nrelu * C:],
                                     Act.Exp)
                ealls.append(eall)
            P['A'] = [[None] * 7 for _ in range(4)]
            P['GtM'] = []
            for h in range(4):
                pls = []
                for csrc in (kb[h], qb[h]):
                    pl = prc.tile([128, 512], fp32, tag="big", name="pall", bufs=2)
                    ph = prc.tile([128, 512], fp32, tag="big", name="pallh", bufs=2)
                    for n in range(8):
                        dst = pl if n < 4 else ph
                        if n in (0, 2, 4):
                            nc.tensor.matmul(dst[:, (n % 4) * C:(n % 4 + 1) * C],
                                             kb[h][16 * n:16 * (n + 1), ts],
                                             csrc[16 * n:16 * (n + 1), ts],
                                             start=True, stop=True)
                        else:
                            j = MSKN[n]
                            nc.tensor.matmul(dst[:, (n % 4) * C:(n % 4 + 1) * C],
                                             kmsks[h][:, j * C:(j + 1) * C],
                                             csrc[:, ts],
                                             start=True, stop=True)
                    pls.append((pl, ph))
                prods = []
                for x, (pl, ph) in enumerate(pls):
                    prod = rr.tile([128, 8 * C], bf16, tag="prod", name=f"prod{x}", bufs=4)
                    nc.vector.tensor_tensor(prod[:, 0:4 * C], ealls[h][:, 0:4 * C],
                                            pl[:], op=Alu.mult)
                    nc.vector.tensor_tensor(prod[:, 4 * C:], ealls[h][:, 4 * C:],
                                            ph[:], op=Alu.mult)
                    prods.append(prod)
                for x, prod in enumerate(prods):
                    # sum the 8 group blocks on PE (identity-lhsT accumulation)
                    t1 = prc.tile([128, C], fp32, tag="dblx", name="t1p", bufs=2)
                    for n in range(8):
                        nc.tensor.matmul(t1[:], idb, prod[:, n * C:(n + 1) * C],
                                         start=(n == 0), stop=(n == 7))
                    if x == 0:
                        A0 = rc.tile([128, C], bf16, tag=f"A{h}", name=f"A{h}_0",
                                     bufs=12)
                        nc.vector.scalar_tensor_tensor(A0[:], t1[:],
                                                       beta2[:, h:h + 1], mMt[:],
                                                       op0=Alu.mult, op1=Alu.mult)
                        P['A'][h][0] = A0
                    else:
                        GtM = hdt("GtM", h, [128, C])
                        nc.vector.scalar_tensor_tensor(GtM[:], t1[:],
                                                       beta2[:, h:h + 1], mGt[:],
                                                       op0=Alu.mult, op1=Alu.mult)
                        P['GtM'].append(GtM)
            P['_kends'] = kends
            P['_beta2'] = beta2
            if with_vt:
                pro_vt(P)
            # A/B doubling chains (state-independent)
            Bs = [[None] * 6 for _ in range(4)]
            for h in range(4):
                b0p = prc.tile([128, C], bf16, tag="tp", name="b0p", bufs=2)
                nc.tensor.transpose(b0p[:], P['A'][h][0][:], idb[:])
                B0 = rc.tile([128, C], bf16, tag=f"B{h}", name=f"B{h}_0", bufs=2)
                nc.scalar.copy(B0[:], b0p[:])
                Bs[h][0] = B0
            for lev in range(1, 7):
                for h in range(4):
                    dbl = prc.tile([128, 256], fp32, tag="dblx", name="dbl", bufs=2)
                    nc.tensor.matmul(dbl[:, 0:128], Bs[h][lev - 1][:],
                                     P['A'][h][lev - 1][:], start=True, stop=True)
                    if lev < 6:
                        nc.tensor.matmul(dbl[:, 128:256], P['A'][h][lev - 1][:],
                                         Bs[h][lev - 1][:], start=True, stop=True)
                    An = rc.tile([128, C], bf16, tag=f"A{h}", name=f"A{h}_{lev}",
                                 bufs=12)
                    nc.scalar.copy(An[:], dbl[:, 0:128])
                    P['A'][h][lev] = An
                    if lev < 6:
                        Bn = rc.tile([128, C], bf16, tag=f"B{h}",
                                     name=f"B{h}_{lev}", bufs=2)
                        if lev % 2 == 0 and not early:
                            nc.scalar.copy(Bn[:], dbl[:, 128:256])
                        else:
                            nc.vector.tensor_copy(Bn[:], dbl[:, 128:256])
                        Bs[h][lev] = Bn
            return P

        def spine(ci, P):
            ts = P['ts']
            po, pn = ci % 2, (ci + 1) % 2
            xbs = []
            for h in range(4):
                ws0 = prc.tile([128, C], fp32, tag="tp", name="ws0", bufs=2)
                nc.tensor.matmul(ws0[:], P['Wt'][h][:], Sb[h][po][:],
                                 start=True, stop=True)
                xb = rc.tile([128, C], bf16, tag=f"xb{h}", name=f"xb{h}", bufs=3)
                nc.vector.tensor_tensor(xb[:], P['vt'][h][:], ws0[:],
                                        op=Alu.subtract)
                xbs.append(xb)
            for lev in range(7):
                for h in range(4):
                    mx = prc.tile([128, C], fp32, tag="mx", name="mx", bufs=2)
                    nc.tensor.matmul(mx[:], P['A'][h][lev][:], xbs[h][:],
                                     start=True, stop=True)
                    xn = rc.tile([128, C], bf16, tag=f"xb{h}", name=f"xb{h}_{lev}",
                                 bufs=3)
                    nc.vector.tensor_tensor(xn[:], xbs[h][:], mx[:],
                                            op=(Alu.subtract if lev == 0 else Alu.add))
                    xbs[h] = xn
            for h in range(4):
                sup = prc.tile([128, DV], fp32, tag="tp", name="sup", bufs=2)
                nc.tensor.matmul(sup[:], P['kts'][h][:], xbs[h][:],
                                 start=True, stop=True)
                nc.vector.scalar_tensor_tensor(Sf[h][pn][:], Sf[h][po][:],
                                               P['bC'][h][:, 0:1], sup[:],
                                               op0=Alu.mult, op1=Alu.add)
                nc.vector.scalar_tensor_tensor(Sb[h][pn][:], Sf[h][po][:],
                                               P['bC'][h][:, 0:1], sup[:],
                                               op0=Alu.mult, op1=Alu.add)
                otp = prc.tile([128, C], fp32, tag="tp", name="otp", bufs=2)
                nc.tensor.matmul(otp[:], Sb[h][po][:], P['qtT'][h][:],
                                 start=True, stop=False)
                nc.tensor.matmul(otp[:], xbs[h][:], P['GtM'][h][:],
                                 start=False, stop=True)
                nc.vector.tensor_tensor(yb[h][:, ts], gateb[h][:, ts], otp[:],
                                        op=Alu.mult)

        pros = [None, None]
        pros[0] = prologue(0, with_vt=False, early=True)
        pros[1] = prologue(1, with_vt=False, early=True)
        # g projection emitted here: its PE work overlaps prologue 0/1
        for m in range(4):
            project(wtB, prc, 0, m, dst_bf16=gateb[m],
                    gate_bias=bgt[:, m:m + 1], ptag="mx", pbufs=2)
        for k in range(4):
            dma(wot[k][:], wo[k * 128:(k + 1) * 128, :])
        pro_vt(pros[0], early=True)
        pro_vt(pros[1], early=True)
        spine(0, pros[0])
        for ci in range(2, NCH):
            pros[ci % 2] = prologue(ci)
            spine(ci - 1, pros[(ci - 1) % 2])
        spine(NCH - 1, pros[(NCH - 1) % 2])

        rctx.close()
        # ================= deferred RMSNorm + output projection =================
        with tc.tile_pool(name="post", bufs=2) as post, \
             tc.tile_pool(name="ppc", bufs=2, space="PSUM") as ppc:
            # PSUM tags: ssp(2) rbc(2) proj(2) = 6 banks
            ysqs = []
            for h in range(4):
                ysq = post.tile([128, T], bf16, tag="ysq", name="ysq", bufs=4)
                nc.scalar.activation(ysq[:], yb[h][:], Act.Square)
                ysqs.append(ysq)
            nrcs = []
            for h in range(4):
                nrc = post.tile([1, T], fp32, tag="nrc", name="nrc", bufs=4)
                for half in range(2):
                    ssp = ppc.tile([1, 512], fp32, tag="ssp", name="ssp")
                    nc.tensor.matmul(ssp[:], octb[:],
                                     ysqs[h][:, half * 512:(half + 1) * 512],
                                     start=True, stop=True)
                    nc.scalar.activation(nrc[:, half * 512:(half + 1) * 512],
                                         ssp[:], Act.Ln, scale=1.0 / DV,
                                         bias=epsnt[:, 0:1])
                nrcs.append(nrc)
            for h in range(4):
                rcb = post.tile([1, T], bf16, tag="rcb", name="rcb", bufs=4)
                nc.scalar.activation(rcb[:], nrcs[h][:], Act.Exp, scale=-0.5)
                for half in range(2):
                    rbc = ppc.tile([128, 512], fp32, tag="rbc", name="rbc")
                    nc.tensor.matmul(rbc[:], o1b[:], rcb[:, half * 512:(half + 1) * 512],
                                     start=True, stop=True)
                    nc.vector.scalar_tensor_tensor(yb[h][:, half * 512:(half + 1) * 512],
                                                   yb[h][:, half * 512:(half + 1) * 512],
                                                   nwt[:, 0:1], rbc[:],
                                                   op0=Alu.mult, op1=Alu.mult)
            # output projection
            for m in range(16):
                osb = post.tile([128, T], fp32, tag="osb", name="osb")
                for half in range(2):
                    ps = ppc.tile([128, 512], fp32, tag="proj", name="ops")
                    for k in range(4):
                        nc.tensor.matmul(ps[:], wot[k][:, m * 128:(m + 1) * 128],
                                         yb[k][:, half * 512:(half + 1) * 512],
                                         start=(k == 0), stop=(k == 3))
                    if half == 0:
                        nc.vector.tensor_copy(osb[:, 0:512], ps[:])
                    else:
                        nc.scalar.copy(osb[:, 512:1024], ps[:])
                dma(outT[m * 128:(m + 1) * 128, :], osb[:])

    nc.compile()
    return nc


def _prep_inputs(inputs):
    f32 = np.float32
    hs = np.asarray(inputs['hidden_states'], f32)
    tri = np.tril(np.ones((C, C), f32))
    maskM = (1.0 - tri).astype(f32)
    maskG = (1.0 - tri + np.eye(C, dtype=f32)).astype(f32)
    repl = np.zeros((NG, DK), f32)
    for n in range(NG):
        repl[n, n * GG:(n + 1) * GG] = 1.0
    sel8 = np.zeros((NG, NG * 128), f32)
    for n in range(NG):
        sel8[n, n * 128:(n + 1) * 128] = 1.0
    oh8 = np.zeros((DK, 64), f32)
    for i in range(8):
        oh8[:, i * 8 + i] = 1.0
    ident = np.eye(128, dtype=f32)

    maps = []
    for c in range(8):
        b, hg = c // 4, c % 4
        cols = slice(hg * NH * DK, (hg + 1) * NH * DK)
        gcols = slice(hg * NH * NG, (hg + 1) * NH * NG)
        hcols = slice(hg * NH, (hg + 1) * NH)
        nega = -np.exp(np.repeat(np.asarray(inputs['A_log'], f32)[hcols], NG))

        packf = np.zeros((128, NF), f32)
        cw = np.concatenate(
            [np.asarray(inputs['conv_q'], f32)[cols],
             np.asarray(inputs['conv_k'], f32)[cols],
             np.asarray(inputs['conv_v'], f32)[cols]], 1)  # [512, 12]
        for m in range(4):
            packf[:, PF_CW + m * 12:PF_CW + (m + 1) * 12] = cw[m * 128:(m + 1) * 128]
        packf[:, PF_BG:PF_BG + 4] = np.asarray(inputs['bg'], f32)[cols].reshape(NH, DV).T
        packf[:, PF_NW] = np.asarray(inputs['norm_w'], f32)
        packf[0:8, PF_NEGA:PF_NEGA + 4] = nega.reshape(NH, NG).T
        packf[0:8, PF_DTB:PF_DTB + 4] = (
            np.asarray(inputs['dt_bias'], f32)[gcols].reshape(NH, NG).T)
        packf[0:8, PF_SC8] = [1.0 / SCALE ** 2] * 4 + [1.0] * 4
        packf[0:8, PF_EPS8] = [1e-6 / SCALE ** 2] * 4 + [1e-6] * 4
        packf[0:1, PF_EPSN] = EPS
        packf[0:8, PF_REPL:PF_REPL + 128] = repl
        packf[:, PF_IDF:PF_IDF + 128] = ident
        packf[0:8, PF_SEL:PF_SEL + 1024] = sel8
        packf[:, PF_GMC:PF_GMC + 8] = repl.T

        packb = np.zeros((128, NB), f32)
        packb[:, PB_OH8:PB_OH8 + 64] = oh8
        packb[0:8, PB_S8B:PB_S8B + 1024] = sel8
        packb[:, PB_MM:PB_MM + 128] = maskM
        packb[:, PB_MG:PB_MG + 128] = maskG
        packb[:, PB_IDB:PB_IDB + 128] = ident
        packb[:, PB_OCT] = 1.0
        packb[0:1, PB_O1B:PB_O1B + 128] = 1.0

        wallm = np.concatenate(
            [np.asarray(inputs['Wq'], f32)[:, cols],
             np.asarray(inputs['Wk'], f32)[:, cols],
             np.asarray(inputs['Wv'], f32)[:, cols],
             np.asarray(inputs['Wg'], f32)[:, cols],
             np.asarray(inputs['Wf1'], f32),
             np.asarray(inputs['Wb'], f32)[:, hcols]], 1)

        m = {
            'hT': np.ascontiguousarray(hs[b].T).astype(BF),
            'wall': np.ascontiguousarray(wallm).astype(BF),
            'wo': np.ascontiguousarray(np.asarray(inputs['Wo'], f32)[cols, :]).astype(BF),
            'wf2': np.ascontiguousarray(np.asarray(inputs['Wf2'], f32)[:, gcols]).astype(BF),
            'packf': packf,
            'packb': packb.astype(BF),
        }
        maps.append(m)
    return maps


def kernel(**inputs):
    from concourse.bass_utils import run_bass_kernel_spmd
    if 'nc' not in _CACHE:
        _CACHE['nc'] = _build()
    nc = _CACHE['nc']
    maps = _prep_inputs(inputs)
    res = run_bass_kernel_spmd(nc, maps, list(range(8))).results
    out = np.zeros((B, T, D), np.float32)
    for c in range(8):
        out[c // 4] += res[c]['outT'].T.astype(np.float32)
    return out
